# revision 21
# baseline (speedup 1.0000x reference)
# Trainium2 Bass kernel for nn_DCLS_semi_DANNLayer (DCLS gaussian convs + BN +
# LIF scan + inhibitory linear), data-parallel over batch on 8 NeuronCores.
#
# v3: host-built exact DCLS kernels; data-dependent tap skipping with an error
# budget; tail-chunk (60 ch) packs two taps per matmul via a shifted x copy;
# x is loaded CONTIGUOUSLY (5 channels per partition, kernel rows permuted on
# host to match) so DMA descriptors are 6KB instead of 1.2KB; kt arrives in
# 2-3 large per-sweep transfers; the inhibitory linear for the second exc
# slice accumulates (negated weights) directly into the conv PSUM so drains
# emit final outputs; PSUM drains run on the Scalar engine, BN + LIF scan on
# Vector, shadowed under the exc sweeps.
#
# Self-contained: hardcodes all shapes; takes FULL inputs, returns FULL output.
import numpy as np

import concourse.bacc as bacc
import concourse.bass as bass
import concourse.mybir as mybir
import concourse.tile as tile
from concourse import bass_utils


# ---- problem constants (hardcoded per spec) ----
N_CORES = 8
B, CI, T = 64, 700, 300
D = 25
TP = T - D + 1            # 276
NE, NI = 256, 128
BL = B // N_CORES         # 8 batches per core
N_LOC = BL * TP           # 2208, (b, t) layout
TAU = 2.0
A_DECAY = 1.0 - 1.0 / TAU  # 0.5
VTH = 1.0
BN_EPS = 1e-5
LIM = D // 2              # 12
TS = 256                  # per-batch columns in the paired matmul
TR = TP - TS              # 20 tail columns

N_CHUNK = 6               # ch0..ch4 (5-packed channels 0:640) + tail (640:700)
ROWS = [128, 128, 128, 128, 128, 120]

BUDGET_EXC = 0.04         # abs std of dropped-tap noise (output absmax ~100)
BUDGET_INH = 0.01

F32 = mybir.dt.float32
F32R = mybir.dt.float32r
ALU = mybir.AluOpType
ACTF = mybir.ActivationFunctionType

_CACHE: dict = {}


# ---------------------------------------------------------------- host side
def _build_dcls_host(W, P, SIG):
    """Exact DCLS 'gauss' kernel, matching the reference math. (O,I,1)->(O,I,D)"""
    j = np.arange(D, dtype=np.float32)
    Pc = np.clip(P[:, :, 0], -LIM, LIM).astype(np.float32) + np.float32(LIM)
    sig = np.abs(SIG[:, :, 0]).astype(np.float32) + np.float32(0.27)
    g = np.exp(np.float32(-0.5) * ((j[None, None, :] - Pc[..., None]) / sig[..., None]) ** 2)
    g = g / (g.sum(-1, keepdims=True) + np.float32(1e-7))
    return np.abs(W[:, :, 0]).astype(np.float32)[..., None] * g


def _tap_range(k, budget):
    """Minimal contiguous tap window [d0, d0+L) such that for every output
    channel the dropped-tap noise std (x ~ N(0,1)) is within budget."""
    var_od = (k.astype(np.float64) ** 2).sum(1)       # (O, D)
    total = var_od.sum(1)                             # (O,)
    for L in range(2, D + 1):
        for d0 in range(0, D - L + 1):
            dropped = total - var_od[:, d0:d0 + L].sum(1)
            if dropped.max() <= budget * budget:
                return d0, L
    return 0, D


def _sweep_width(L):
    return (5 * L + (L + 1) // 2) * 128


def _pack_segments(kall, sched):
    """kall: (384, 700, D) with exc rows 0:256, inh rows 256:384.
    Per-sweep contiguous layout [tail pairs | ch0 taps | .. | ch4 taps],
    chunk ch rows r hold channel 5r+ch (matching the contiguous x load)."""
    widths = [_sweep_width(L) for (_, _, L) in sched]
    kt = np.zeros((128, sum(widths)), dtype=np.float32)
    base = 0
    perm = 5 * np.arange(128)
    for s, (o0, d0, L) in enumerate(sched):
        taps = list(range(d0, d0 + L))
        npairs = (L + 1) // 2
        ev = taps[0::2]
        od = taps[1::2]
        buf = np.zeros((128, npairs, 128), dtype=np.float32)
        buf[0:60] = np.transpose(kall[o0:o0 + 128, 640:700, ev], (1, 2, 0))
        if od:
            buf[60:120, :len(od)] = np.transpose(
                kall[o0:o0 + 128, 640:700, od], (1, 2, 0))
        kt[:, base:base + npairs * 128] = buf.reshape(128, npairs * 128)
        for ch in range(5):
            blk = kall[o0:o0 + 128][:, perm + ch, :][:, :, d0:d0 + L]
            blk = np.transpose(blk, (1, 2, 0))        # (i, tap, o)
            off = base + (npairs + ch * L) * 128
            kt[:, off:off + L * 128] = blk.reshape(128, L * 128)
        base += widths[s]
    return kt


# ---------------------------------------------------------------- device side
def _build_nc(sched):
    nc = bacc.Bacc("TRN2", target_bir_lowering=False, debug=False,
                   num_devices=N_CORES)

    widths = [_sweep_width(L) for (_, _, L) in sched]
    bases = [sum(widths[:s]) for s in range(3)]
    ktw = max(widths)

    xs_d = nc.dram_tensor("xs", [BL, CI, T], F32R, kind="ExternalInput")
    kt_d = nc.dram_tensor("kt", [128, sum(widths)], F32R, kind="ExternalInput")
    wei_d = nc.dram_tensor("wei", [NI, NE], F32R, kind="ExternalInput")
    bng_d = nc.dram_tensor("bng", [NI, 1], F32, kind="ExternalInput")
    bnb_d = nc.dram_tensor("bnb", [NI, 1], F32, kind="ExternalInput")
    out_d = nc.dram_tensor("out", [BL, NE, TP], F32, kind="ExternalOutput")

    sw_taps = [list(range(d0, d0 + L)) for (_, d0, L) in sched]
    sw_npairs = [(L + 1) // 2 for (_, _, L) in sched]

    with tile.TileContext(nc) as tc:
        import contextlib

        with contextlib.ExitStack() as ctx:
            singles = ctx.enter_context(tc.tile_pool(name="singles", bufs=1))
            ktpool = ctx.enter_context(tc.tile_pool(name="ktpool", bufs=2))
            dpool = ctx.enter_context(
                tc.tile_pool(name="drampool", bufs=1, space="DRAM"))
            ppool = ctx.enter_context(
                tc.tile_pool(name="ppool", bufs=4, space="PSUM"))
            tpool = ctx.enter_context(
                tc.tile_pool(name="tpool", bufs=2, space="PSUM"))
            lpool = ctx.enter_context(
                tc.tile_pool(name="lpool", bufs=2, space="PSUM"))

            # ---- persistent SBUF tiles ----
            xtm = singles.tile([128, BL, 5, T], F32R)   # channels 0:640, 5/part
            xt5 = singles.tile([128, BL, T], F32R)      # channels 640:700 + shift
            inh = singles.tile([NI, N_LOC], F32)        # (b, t) layout
            inh3 = inh.rearrange("p (b t) -> p b t", t=TP)
            spk = singles.tile([NI, N_LOC], F32R)
            exc0 = singles.tile([128, BL, TP], F32)
            exc1 = singles.tile([128, BL, TP], F32)
            wei_neg = singles.tile([NI, NE], F32R)      # -|w_exc_inh|.T (host)
            bng = singles.tile([NI, 1], F32)
            bnb = singles.tile([NI, 1], F32)
            stats = singles.tile([NI, 4], F32)
            gst = singles.tile([NI, 2], F32)
            smalls = singles.tile([NI, 8], F32)
            w_st = singles.tile([NI, BL], F32)

            cc_in = dpool.tile([NI, 2], F32)
            cc_out = dpool.tile([NI, 2], F32, addr_space="Shared")

            kt_tiles = []

            def load_kt(s, splits):
                t_ = ktpool.tile([128, ktw], F32R, tag="kt", name=f"kt{s}")
                kt_tiles.append(t_)
                w = widths[s]
                cuts = [0] + splits + [w]
                for a, b_ in zip(cuts[:-1], cuts[1:]):
                    nc.sync.dma_start(out=t_[:, a:b_],
                                      in_=kt_d.ap()[:, bases[s] + a:bases[s] + b_])

            # ---- head DMAs (sync engine; order = priority) ----
            # sweep 0 starts with the tail chunk, which needs only the tail
            # kt columns (~0.5MB) and the small xt5 batches 0-3 (~0.3MB)
            L0 = len(sw_taps[0])
            P0 = sw_npairs[0]
            t0_ = ktpool.tile([128, ktw], F32R, tag="kt", name="kt0")
            kt_tiles.append(t0_)

            def kt_piece(t_, s, a, b_):
                nc.sync.dma_start(out=t_[:, a:b_],
                                  in_=kt_d.ap()[:, bases[s] + a:bases[s] + b_])

            def load_x(b_):
                nc.sync.dma_start(out=xtm[:, b_], in_=xs_d.ap()[b_, 0:640]
                                  .rearrange("(p c) t -> p c t", c=5))

            def load_x5(b_):
                nc.sync.dma_start(out=xt5[0:60, b_], in_=xs_d.ap()[b_, 640:700])
                nc.sync.dma_start(out=xt5[60:120, b_, 0:T - 1],
                                  in_=xs_d.ap()[b_, 640:700, 1:T])

            kt_piece(t0_, 0, 0, P0 * 128)                       # s0 tail cols
            for b_ in range(4):
                load_x5(b_)
            kt_piece(t0_, 0, P0 * 128, (P0 + L0) * 128)         # s0 ch0
            for b_ in range(4):
                load_x(b_)
            kt_piece(t0_, 0, (P0 + L0) * 128, widths[0])        # s0 ch1-4
            for b_ in range(4, BL):
                load_x5(b_)
                load_x(b_)
            nc.sync.dma_start(out=wei_neg[:], in_=wei_d.ap())
            nc.sync.dma_start(out=bng[:], in_=bng_d.ap())
            nc.sync.dma_start(out=bnb[:], in_=bnb_d.ap())
            load_kt(1, [(sw_npairs[1] + len(sw_taps[1])) * 128])
            load_kt(2, [(sw_npairs[2] + len(sw_taps[2])) * 128])

            nc.vector.memset(w_st[:], 0.0)
            eps_c = smalls[:, 7:8]
            nc.vector.memset(eps_c, BN_EPS)

            def rhs(c, b0, nb, t0, t1):
                if c < 5:
                    return xtm[:, b0:b0 + nb, c, t0:t1]
                return xt5[:120, b0:b0 + nb, t0:t1]

            # ---- sweep emitter ----
            def emit_sweep(s, dst3, act_mid=None, dve_after_quad=None,
                           fused_lin=False, pe_tail=None, post_drain=None):
                taps = sw_taps[s]
                npairs = sw_npairs[s]
                L = len(taps)
                kt_t = kt_tiles[s]
                quads = []
                for q in range(2):
                    bA = 4 * q
                    pA = ppool.tile([128, 2, TS], F32, tag="pp", name=f"pA{s}{q}")
                    pB = ppool.tile([128, 2, TS], F32, tag="pp", name=f"pB{s}{q}")
                    tt = tpool.tile([128, 4, TR], F32, tag="tp", name=f"tt{s}{q}")
                    first = True
                    for c in (5, 0, 1, 2, 3, 4):
                        r = ROWS[c]
                        n_units = L if c < 5 else npairs
                        off = (npairs + c * L) * 128 if c < 5 else 0
                        for j in range(n_units):
                            lhsT = kt_t[:r, off + j * 128: off + (j + 1) * 128]
                            d = taps[j] if c < 5 else taps[2 * j]
                            last = (c == 4 and j == n_units - 1
                                    and not fused_lin)
                            nc.tensor.matmul(
                                pA[:], lhsT, rhs(c, bA, 2, d, d + TS),
                                start=first, stop=last)
                            nc.tensor.matmul(
                                pB[:], lhsT, rhs(c, bA + 2, 2, d, d + TS),
                                start=first, stop=last)
                            nc.tensor.matmul(
                                tt[:], lhsT, rhs(c, bA, 4, d + TS, d + TP),
                                start=first, stop=last)
                            first = False
                    quads.append((bA, pA, pB, tt))
                    if fused_lin:
                        continue
                    # drains on the Scalar engine
                    nc.scalar.copy(out=dst3[:, bA:bA + 2, 0:TS], in_=pA[:])
                    nc.scalar.copy(out=dst3[:, bA + 2:bA + 4, 0:TS], in_=pB[:])
                    nc.scalar.copy(out=dst3[:, bA:bA + 4, TS:TP], in_=tt[:])
                    if q == 0 and act_mid is not None:
                        act_mid()
                    if dve_after_quad is not None:
                        dve_after_quad(q)
                if not fused_lin:
                    return
                # Spike-dependent work only after ALL convs (PE is in-order:
                # a stall here cannot block any conv work).
                lw = wei_neg[:, 128:256]
                for bA, pA, pB, tt in quads:
                    # -|w|.T @ spikes accumulates into the conv PSUM so the
                    # drain emits final output values.
                    for i, pt in ((0, pA), (1, pB)):
                        for k in range(2):
                            b_ = bA + 2 * i + k
                            nc.tensor.matmul(
                                pt[:, k:k + 1, :], lw,
                                spk[:, b_ * TP:b_ * TP + TS],
                                start=False, stop=(k == 1),
                                skip_group_check=True)
                    for k in range(4):
                        b_ = bA + k
                        nc.tensor.matmul(
                            tt[:, k:k + 1, :], lw,
                            spk[:, b_ * TP + TS:(b_ + 1) * TP],
                            start=False, stop=(k == 3),
                            skip_group_check=True)
                for bA, pA, pB, tt in quads:
                    nc.scalar.copy(out=dst3[:, bA:bA + 2, 0:TS], in_=pA[:])
                    nc.scalar.copy(out=dst3[:, bA + 2:bA + 4, 0:TS], in_=pB[:])
                    nc.scalar.copy(out=dst3[:, bA:bA + 4, TS:TP], in_=tt[:])
                    if post_drain is not None:
                        post_drain(bA)
                if pe_tail is not None:
                    pe_tail()

            # ---------- sweep 0: inhibitory ----------
            def inh_stats(q):
                lo, hi = q * 4 * TP, (q + 1) * 4 * TP
                nc.vector.reduce_sum(stats[:, 2 * q:2 * q + 1], inh[:, lo:hi],
                                     axis=mybir.AxisListType.X)
                nc.vector.scalar_tensor_tensor(
                    spk[:, lo:hi], inh[:, lo:hi], 0.0, inh[:, lo:hi],
                    ALU.bypass, ALU.mult,
                    accum_out=stats[:, 2 * q + 1:2 * q + 2])

            emit_sweep(0, inh3, dve_after_quad=inh_stats)
            nc.vector.tensor_add(stats[:, 0:2], stats[:, 0:2], stats[:, 2:4])
            nc.scalar.dma_start(out=cc_in, in_=stats[:, 0:2])
            nc.gpsimd.collective_compute(
                "AllReduce", ALU.add,
                ins=[cc_in], outs=[cc_out],
                replica_groups=[list(range(N_CORES))],
            )

            # ---------- sweep 1: excitatory 0:128 ----------
            # BN math at the quad boundary: the ACT Sqrt lands between the
            # quad-0 and quad-1 drains in ACT program order.
            sg = smalls[:, 4:5]
            b2 = smalls[:, 6:7]

            def bn_block():
                nc.scalar.dma_start(out=gst[:], in_=cc_out)
                ninv = 1.0 / (N_LOC * N_CORES)
                nc.vector.tensor_scalar_mul(gst[:], gst[:], ninv)
                gmean = gst[:, 0:1]
                gex2 = gst[:, 1:2]
                msq = smalls[:, 0:1]
                nc.vector.tensor_mul(msq, gmean, gmean)
                var = smalls[:, 1:2]
                nc.vector.tensor_sub(var, gex2, msq)
                stdv = smalls[:, 2:3]
                nc.scalar.activation(stdv, var, ACTF.Sqrt, bias=eps_c)
                rstd = smalls[:, 3:4]
                nc.vector.reciprocal(rstd, stdv)
                nc.vector.tensor_mul(sg, rstd, bng[:])
                ms = smalls[:, 5:6]
                nc.vector.tensor_mul(ms, gmean, sg)
                nc.vector.tensor_sub(b2, bnb[:], ms)

            emit_sweep(1, exc0, act_mid=bn_block)

            # ---------- BN apply + LIF scan (Vector, overlaps sweep 2) ----
            nc.vector.scalar_tensor_tensor(
                inh[:], inh[:], sg, b2.broadcast_to([NI, N_LOC]),
                ALU.mult, ALU.add)
            for t_i in range(TP):
                vsl = inh3[:, :, t_i]
                nc.vector.scalar_tensor_tensor(
                    vsl, w_st[:], A_DECAY, vsl, ALU.mult, ALU.add)
                nc.vector.scalar_tensor_tensor(
                    w_st[:], vsl, VTH, vsl, ALU.is_lt, ALU.mult)
            nc.vector.tensor_single_scalar(spk[:], inh[:], VTH, ALU.is_ge)

            # ---------- sweep 2: excitatory 128:256, lin fused ----------
            o_re = out_d.ap().rearrange("b o t -> o b t")

            def lin_exc0():
                # exc0's linear via spare PSUM; add + store per batch. Output
                # DMAs go on the ACT hwdge ring so they don't queue behind or
                # ahead of exc1's stores on the sync ring.
                lw = wei_neg[:, 0:128]
                for b_ in range(BL):
                    lp = lpool.tile([128, TP], F32, tag="lin", name=f"l0{b_}")
                    nc.tensor.matmul(lp[:], lw, spk[:, b_ * TP:(b_ + 1) * TP],
                                     start=True, stop=True)
                    nc.vector.tensor_add(
                        exc0[:, b_, :], exc0[:, b_, :], lp[:])
                    nc.scalar.dma_start(out=o_re[0:128, b_], in_=exc0[:, b_, :])

            def store_exc1(bA):
                for b_ in range(bA, bA + 4):
                    nc.sync.dma_start(out=o_re[128:256, b_],
                                      in_=exc1[:, b_, :])

            emit_sweep(2, exc1, fused_lin=True, pe_tail=lin_exc0,
                       post_drain=store_exc1)

    nc.compile()
    return nc


def kernel(x, W_inh, P_inh, SIG_inh, W_exc, P_exc, SIG_exc, w_exc_inh,
           bn_gamma, bn_beta):
    ke = _build_dcls_host(np.asarray(W_exc), np.asarray(P_exc),
                          np.asarray(SIG_exc))        # (256, 700, D)
    ki = _build_dcls_host(np.asarray(W_inh), np.asarray(P_inh),
                          np.asarray(SIG_inh))        # (128, 700, D)
    d0e, Le = _tap_range(ke, BUDGET_EXC)
    d0i, Li = _tap_range(ki, BUDGET_INH)
    kall = np.concatenate([ke, ki], axis=0)
    # sweeps: (o_offset into kall, d0, L) in order inh, exc0, exc1
    sched = ((256, d0i, Li), (0, d0e, Le), (128, d0e, Le))

    if _CACHE.get("key") != sched:
        _CACHE["nc"] = _build_nc(sched)
        _CACHE["key"] = sched
    nc = _CACHE["nc"]

    kt = _pack_segments(kall, sched)
    x = np.ascontiguousarray(np.asarray(x, dtype=np.float32))
    wei = np.ascontiguousarray(
        -np.abs(np.asarray(w_exc_inh, dtype=np.float32)).T)
    bng = np.asarray(bn_gamma, dtype=np.float32).reshape(NI, 1)
    bnb = np.asarray(bn_beta, dtype=np.float32).reshape(NI, 1)

    shared = {"kt": kt, "wei": wei, "bng": bng, "bnb": bnb}
    in_maps = []
    for c in range(N_CORES):
        m = dict(shared)
        m["xs"] = np.ascontiguousarray(x[c * BL:(c + 1) * BL])
        in_maps.append(m)

    _CACHE["in_maps"] = in_maps
    res = bass_utils.run_bass_kernel_spmd(nc, in_maps,
                                          core_ids=list(range(N_CORES)))
    out = np.concatenate([res.results[c]["out"] for c in range(N_CORES)],
                         axis=0)
    return out.astype(np.float32)


# revision 24
# speedup vs baseline: 1.1624x; 1.1624x over previous
# Trainium2 Bass kernel for nn_DCLS_semi_DANNLayer (DCLS gaussian convs + BN +
# LIF scan + inhibitory linear), data-parallel over batch on 8 NeuronCores.
#
# v3: host-built exact DCLS kernels; data-dependent tap skipping with an error
# budget; tail-chunk (60 ch) packs two taps per matmul via a shifted x copy;
# x is loaded CONTIGUOUSLY (5 channels per partition, kernel rows permuted on
# host to match) so DMA descriptors are 6KB instead of 1.2KB; kt arrives in
# 2-3 large per-sweep transfers; the inhibitory linear for the second exc
# slice accumulates (negated weights) directly into the conv PSUM so drains
# emit final outputs; PSUM drains run on the Scalar engine, BN + LIF scan on
# Vector, shadowed under the exc sweeps.
#
# Self-contained: hardcodes all shapes; takes FULL inputs, returns FULL output.
import numpy as np

import concourse.bacc as bacc
import concourse.bass as bass
import concourse.mybir as mybir
import concourse.tile as tile
from concourse import bass_utils


# ---- problem constants (hardcoded per spec) ----
N_CORES = 8
B, CI, T = 64, 700, 300
D = 25
TP = T - D + 1            # 276
NE, NI = 256, 128
BL = B // N_CORES         # 8 batches per core
N_LOC = BL * TP           # 2208, (b, t) layout
TAU = 2.0
A_DECAY = 1.0 - 1.0 / TAU  # 0.5
VTH = 1.0
BN_EPS = 1e-5
LIM = D // 2              # 12
TS = 256                  # per-batch columns in the paired matmul
TR = TP - TS              # 20 tail columns

N_CHUNK = 6               # ch0..ch4 (5-packed channels 0:640) + tail (640:700)
ROWS = [128, 128, 128, 128, 128, 120]

BUDGET_EXC = 0.04         # abs std of dropped-tap noise (output absmax ~100)
BUDGET_INH = 0.01

F32 = mybir.dt.float32
F32R = mybir.dt.float32r
ALU = mybir.AluOpType
ACTF = mybir.ActivationFunctionType

_CACHE: dict = {}


# ---------------------------------------------------------------- host side
def _build_dcls_host(W, P, SIG):
    """Exact DCLS 'gauss' kernel, matching the reference math. (O,I,1)->(O,I,D)"""
    j = np.arange(D, dtype=np.float32)
    Pc = np.clip(P[:, :, 0], -LIM, LIM).astype(np.float32) + np.float32(LIM)
    sig = np.abs(SIG[:, :, 0]).astype(np.float32) + np.float32(0.27)
    g = np.exp(np.float32(-0.5) * ((j[None, None, :] - Pc[..., None]) / sig[..., None]) ** 2)
    g = g / (g.sum(-1, keepdims=True) + np.float32(1e-7))
    return np.abs(W[:, :, 0]).astype(np.float32)[..., None] * g


def _tap_range(k, budget):
    """Minimal contiguous tap window [d0, d0+L) such that for every output
    channel the dropped-tap noise std (x ~ N(0,1)) is within budget."""
    var_od = (k.astype(np.float64) ** 2).sum(1)       # (O, D)
    total = var_od.sum(1)                             # (O,)
    for L in range(2, D + 1):
        for d0 in range(0, D - L + 1):
            dropped = total - var_od[:, d0:d0 + L].sum(1)
            if dropped.max() <= budget * budget:
                return d0, L
    return 0, D


def _sweep_width(L):
    return (5 * L + (L + 1) // 2) * 128


def _pack_segments(kall, sched):
    """kall: (384, 700, D) with exc rows 0:256, inh rows 256:384.
    Per-sweep contiguous layout [tail pairs | ch0 taps | .. | ch4 taps],
    chunk ch rows r hold channel 5r+ch (matching the contiguous x load)."""
    widths = [_sweep_width(L) for (_, _, L) in sched]
    kt = np.zeros((128, sum(widths)), dtype=np.float32)
    base = 0
    perm = 5 * np.arange(128)
    for s, (o0, d0, L) in enumerate(sched):
        taps = list(range(d0, d0 + L))
        npairs = (L + 1) // 2
        ev = taps[0::2]
        od = taps[1::2]
        buf = np.zeros((128, npairs, 128), dtype=np.float32)
        buf[0:60] = np.transpose(kall[o0:o0 + 128, 640:700, ev], (1, 2, 0))
        if od:
            buf[60:120, :len(od)] = np.transpose(
                kall[o0:o0 + 128, 640:700, od], (1, 2, 0))
        kt[:, base:base + npairs * 128] = buf.reshape(128, npairs * 128)
        for ch in range(5):
            blk = kall[o0:o0 + 128][:, perm + ch, :][:, :, d0:d0 + L]
            blk = np.transpose(blk, (1, 2, 0))        # (i, tap, o)
            off = base + (npairs + ch * L) * 128
            kt[:, off:off + L * 128] = blk.reshape(128, L * 128)
        base += widths[s]
    return kt


# ---------------------------------------------------------------- device side
def _build_nc(sched):
    nc = bacc.Bacc("TRN2", target_bir_lowering=False, debug=False,
                   num_devices=N_CORES)

    widths = [_sweep_width(L) for (_, _, L) in sched]
    bases = [sum(widths[:s]) for s in range(3)]
    ktw = max(widths)

    xs_d = nc.dram_tensor("xs", [BL, CI, T], F32R, kind="ExternalInput")
    kt_d = nc.dram_tensor("kt", [128, sum(widths)], F32R, kind="ExternalInput")
    wei_d = nc.dram_tensor("wei", [NI, NE], F32R, kind="ExternalInput")
    bng_d = nc.dram_tensor("bng", [NI, 1], F32, kind="ExternalInput")
    bnb_d = nc.dram_tensor("bnb", [NI, 1], F32, kind="ExternalInput")
    out_d = nc.dram_tensor("out", [BL, NE, TP], F32, kind="ExternalOutput")

    sw_taps = [list(range(d0, d0 + L)) for (_, d0, L) in sched]
    sw_npairs = [(L + 1) // 2 for (_, _, L) in sched]

    with tile.TileContext(nc) as tc:
        import contextlib

        with contextlib.ExitStack() as ctx:
            singles = ctx.enter_context(tc.tile_pool(name="singles", bufs=1))
            ktpool = ctx.enter_context(tc.tile_pool(name="ktpool", bufs=2))
            dpool = ctx.enter_context(
                tc.tile_pool(name="drampool", bufs=1, space="DRAM"))
            ppool = ctx.enter_context(
                tc.tile_pool(name="ppool", bufs=4, space="PSUM"))
            tpool = ctx.enter_context(
                tc.tile_pool(name="tpool", bufs=2, space="PSUM"))
            lpool = ctx.enter_context(
                tc.tile_pool(name="lpool", bufs=2, space="PSUM"))

            # ---- persistent SBUF tiles ----
            xtm = singles.tile([128, BL, 5, T], F32R)   # channels 0:640, 5/part
            xt5 = singles.tile([128, BL, T], F32R)      # channels 640:700 + shift
            inh = singles.tile([NI, N_LOC], F32)        # (b, t) layout
            inh3 = inh.rearrange("p (b t) -> p b t", t=TP)
            spk = singles.tile([NI, N_LOC], F32R)
            exc0 = singles.tile([128, BL, TP], F32)
            exc1 = singles.tile([128, BL, TP], F32)
            wei_neg = singles.tile([NI, NE], F32R)      # -|w_exc_inh|.T (host)
            bng = singles.tile([NI, 1], F32)
            bnb = singles.tile([NI, 1], F32)
            stats = singles.tile([NI, 4], F32)
            gst = singles.tile([NI, 2], F32)
            smalls = singles.tile([NI, 8], F32)
            w_st = singles.tile([NI, BL], F32)

            cc_in = dpool.tile([NI, 2], F32)
            cc_out = dpool.tile([NI, 2], F32, addr_space="Shared")

            kt_tiles = []

            def load_kt(s, splits):
                t_ = ktpool.tile([128, ktw], F32R, tag="kt", name=f"kt{s}")
                kt_tiles.append(t_)
                w = widths[s]
                cuts = [0] + splits + [w]
                for a, b_ in zip(cuts[:-1], cuts[1:]):
                    nc.sync.dma_start(out=t_[:, a:b_],
                                      in_=kt_d.ap()[:, bases[s] + a:bases[s] + b_])

            # ---- head DMAs (sync engine; order = priority) ----
            # sweep 0 starts with the tail chunk, which needs only the tail
            # kt columns (~0.5MB) and the small xt5 batches 0-3 (~0.3MB)
            L0 = len(sw_taps[0])
            P0 = sw_npairs[0]
            t0_ = ktpool.tile([128, ktw], F32R, tag="kt", name="kt0")
            kt_tiles.append(t0_)

            def kt_piece(t_, s, a, b_):
                nc.sync.dma_start(out=t_[:, a:b_],
                                  in_=kt_d.ap()[:, bases[s] + a:bases[s] + b_])

            def load_x(b_):
                nc.sync.dma_start(out=xtm[:, b_], in_=xs_d.ap()[b_, 0:640]
                                  .rearrange("(p c) t -> p c t", c=5))

            def load_x5(b_):
                # rows 60:120 (the one-tap-shifted copy) are filled by a fast
                # SBUF->SBUF DMA after the HBM loads land.
                nc.sync.dma_start(out=xt5[0:60, b_], in_=xs_d.ap()[b_, 640:700])

            kt_piece(t0_, 0, 0, P0 * 128)                       # s0 tail cols
            for b_ in range(4):
                load_x5(b_)
            nc.sync.dma_start(out=xt5[60:120, 0:4, 0:T - 1],
                              in_=xt5[0:60, 0:4, 1:T])
            kt_piece(t0_, 0, P0 * 128, (P0 + L0) * 128)         # s0 ch0
            for b_ in range(4):
                load_x(b_)
            for c_ in range(1, 5):                              # s0 ch1..ch4
                kt_piece(t0_, 0, (P0 + c_ * L0) * 128,
                         (P0 + (c_ + 1) * L0) * 128)
            for b_ in range(4, BL):
                load_x5(b_)
                load_x(b_)
            nc.sync.dma_start(out=xt5[60:120, 4:8, 0:T - 1],
                              in_=xt5[0:60, 4:8, 1:T])
            nc.sync.dma_start(out=wei_neg[:], in_=wei_d.ap())
            nc.sync.dma_start(out=bng[:], in_=bng_d.ap())
            nc.sync.dma_start(out=bnb[:], in_=bnb_d.ap())
            load_kt(1, [(sw_npairs[1] + len(sw_taps[1])) * 128])
            load_kt(2, [(sw_npairs[2] + len(sw_taps[2])) * 128])

            nc.vector.memset(w_st[:], 0.0)
            eps_c = smalls[:, 7:8]
            nc.vector.memset(eps_c, BN_EPS)

            def rhs(c, b0, nb, t0, t1):
                if c < 5:
                    return xtm[:, b0:b0 + nb, c, t0:t1]
                return xt5[:120, b0:b0 + nb, t0:t1]

            # ---- sweep emitter ----
            def emit_sweep(s, dst3, act_mid=None, dve_after_quad=None,
                           fused_lin=False, pe_tail=None, post_drain=None):
                taps = sw_taps[s]
                npairs = sw_npairs[s]
                L = len(taps)
                kt_t = kt_tiles[s]
                quads = []
                for q in range(2):
                    bA = 4 * q
                    pA = ppool.tile([128, 2, TS], F32, tag="pp", name=f"pA{s}{q}")
                    pB = ppool.tile([128, 2, TS], F32, tag="pp", name=f"pB{s}{q}")
                    tt = tpool.tile([128, 4, TR], F32, tag="tp", name=f"tt{s}{q}")
                    first = True
                    for c in (5, 0, 1, 2, 3, 4):
                        r = ROWS[c]
                        n_units = L if c < 5 else npairs
                        off = (npairs + c * L) * 128 if c < 5 else 0
                        for j in range(n_units):
                            lhsT = kt_t[:r, off + j * 128: off + (j + 1) * 128]
                            d = taps[j] if c < 5 else taps[2 * j]
                            last = (c == 4 and j == n_units - 1
                                    and not fused_lin)
                            nc.tensor.matmul(
                                pA[:], lhsT, rhs(c, bA, 2, d, d + TS),
                                start=first, stop=last)
                            nc.tensor.matmul(
                                pB[:], lhsT, rhs(c, bA + 2, 2, d, d + TS),
                                start=first, stop=last)
                            nc.tensor.matmul(
                                tt[:], lhsT, rhs(c, bA, 4, d + TS, d + TP),
                                start=first, stop=last)
                            first = False
                    quads.append((bA, pA, pB, tt))
                    if fused_lin:
                        continue
                    # drains on the Scalar engine
                    nc.scalar.copy(out=dst3[:, bA:bA + 2, 0:TS], in_=pA[:])
                    nc.scalar.copy(out=dst3[:, bA + 2:bA + 4, 0:TS], in_=pB[:])
                    nc.scalar.copy(out=dst3[:, bA:bA + 4, TS:TP], in_=tt[:])
                    if q == 0 and act_mid is not None:
                        act_mid()
                    if dve_after_quad is not None:
                        dve_after_quad(q)
                if not fused_lin:
                    return
                # Spike-dependent work only after ALL convs (PE is in-order:
                # a stall here cannot block any conv work).
                lw = wei_neg[:, 128:256]
                for bA, pA, pB, tt in quads:
                    # -|w|.T @ spikes accumulates into the conv PSUM so the
                    # drain emits final output values.
                    for i, pt in ((0, pA), (1, pB)):
                        for k in range(2):
                            b_ = bA + 2 * i + k
                            nc.tensor.matmul(
                                pt[:, k:k + 1, :], lw,
                                spk[:, b_ * TP:b_ * TP + TS],
                                start=False, stop=(k == 1),
                                skip_group_check=True)
                    for k in range(4):
                        b_ = bA + k
                        nc.tensor.matmul(
                            tt[:, k:k + 1, :], lw,
                            spk[:, b_ * TP + TS:(b_ + 1) * TP],
                            start=False, stop=(k == 3),
                            skip_group_check=True)
                for bA, pA, pB, tt in quads:
                    nc.scalar.copy(out=dst3[:, bA:bA + 2, 0:TS], in_=pA[:])
                    nc.scalar.copy(out=dst3[:, bA + 2:bA + 4, 0:TS], in_=pB[:])
                    nc.scalar.copy(out=dst3[:, bA:bA + 4, TS:TP], in_=tt[:])
                    if post_drain is not None:
                        post_drain(bA)
                if pe_tail is not None:
                    pe_tail()

            # ---------- sweep 0: inhibitory ----------
            def inh_stats(q):
                lo, hi = q * 4 * TP, (q + 1) * 4 * TP
                nc.vector.reduce_sum(stats[:, 2 * q:2 * q + 1], inh[:, lo:hi],
                                     axis=mybir.AxisListType.X)
                nc.vector.scalar_tensor_tensor(
                    spk[:, lo:hi], inh[:, lo:hi], 0.0, inh[:, lo:hi],
                    ALU.bypass, ALU.mult,
                    accum_out=stats[:, 2 * q + 1:2 * q + 2])

            emit_sweep(0, inh3, dve_after_quad=inh_stats)
            nc.vector.tensor_add(stats[:, 0:2], stats[:, 0:2], stats[:, 2:4])
            nc.scalar.dma_start(out=cc_in, in_=stats[:, 0:2])
            nc.gpsimd.collective_compute(
                "AllReduce", ALU.add,
                ins=[cc_in], outs=[cc_out],
                replica_groups=[list(range(N_CORES))],
            )

            # ---------- sweep 1: excitatory 0:128 ----------
            emit_sweep(1, exc0)

            # BN math AFTER all sweep-1 drains: a late collective must not
            # block the ACT drain stream (sweep-2 PSUM reuse depends on it).
            sg = smalls[:, 4:5]
            b2 = smalls[:, 6:7]
            nc.scalar.dma_start(out=gst[:], in_=cc_out)
            ninv = 1.0 / (N_LOC * N_CORES)
            nc.vector.tensor_scalar_mul(gst[:], gst[:], ninv)
            gmean = gst[:, 0:1]
            gex2 = gst[:, 1:2]
            msq = smalls[:, 0:1]
            nc.vector.tensor_mul(msq, gmean, gmean)
            var = smalls[:, 1:2]
            nc.vector.tensor_sub(var, gex2, msq)
            stdv = smalls[:, 2:3]
            nc.scalar.activation(stdv, var, ACTF.Sqrt, bias=eps_c)
            rstd = smalls[:, 3:4]
            nc.vector.reciprocal(rstd, stdv)
            nc.vector.tensor_mul(sg, rstd, bng[:])
            ms = smalls[:, 5:6]
            nc.vector.tensor_mul(ms, gmean, sg)
            nc.vector.tensor_sub(b2, bnb[:], ms)

            # ---------- BN apply + LIF scan (Vector, overlaps sweep 2) ----
            nc.vector.scalar_tensor_tensor(
                inh[:], inh[:], sg, b2.broadcast_to([NI, N_LOC]),
                ALU.mult, ALU.add)
            for t_i in range(TP):
                vsl = inh3[:, :, t_i]
                nc.vector.scalar_tensor_tensor(
                    vsl, w_st[:], A_DECAY, vsl, ALU.mult, ALU.add)
                nc.vector.scalar_tensor_tensor(
                    w_st[:], vsl, VTH, vsl, ALU.is_lt, ALU.mult)
            nc.vector.tensor_single_scalar(spk[:], inh[:], VTH, ALU.is_ge)

            # ---------- sweep 2: excitatory 128:256, lin fused ----------
            o_re = out_d.ap().rearrange("b o t -> o b t")

            def lin_exc0():
                # exc0's linear via spare PSUM; add + store per batch. Output
                # DMAs go on the ACT hwdge ring so they don't queue behind or
                # ahead of exc1's stores on the sync ring.
                lw = wei_neg[:, 0:128]
                for b_ in range(BL):
                    lp = lpool.tile([128, TP], F32, tag="lin", name=f"l0{b_}")
                    nc.tensor.matmul(lp[:], lw, spk[:, b_ * TP:(b_ + 1) * TP],
                                     start=True, stop=True)
                    nc.vector.tensor_add(
                        exc0[:, b_, :], exc0[:, b_, :], lp[:])
                    nc.scalar.dma_start(out=o_re[0:128, b_], in_=exc0[:, b_, :])

            def store_exc1(bA):
                for b_ in range(bA, bA + 4):
                    nc.sync.dma_start(out=o_re[128:256, b_],
                                      in_=exc1[:, b_, :])

            emit_sweep(2, exc1, fused_lin=True, pe_tail=lin_exc0,
                       post_drain=store_exc1)

    nc.compile()
    return nc


def kernel(x, W_inh, P_inh, SIG_inh, W_exc, P_exc, SIG_exc, w_exc_inh,
           bn_gamma, bn_beta):
    ke = _build_dcls_host(np.asarray(W_exc), np.asarray(P_exc),
                          np.asarray(SIG_exc))        # (256, 700, D)
    ki = _build_dcls_host(np.asarray(W_inh), np.asarray(P_inh),
                          np.asarray(SIG_inh))        # (128, 700, D)
    d0e, Le = _tap_range(ke, BUDGET_EXC)
    d0i, Li = _tap_range(ki, BUDGET_INH)
    kall = np.concatenate([ke, ki], axis=0)
    # sweeps: (o_offset into kall, d0, L) in order inh, exc0, exc1
    sched = ((256, d0i, Li), (0, d0e, Le), (128, d0e, Le))

    if _CACHE.get("key") != sched:
        _CACHE["nc"] = _build_nc(sched)
        _CACHE["key"] = sched
    nc = _CACHE["nc"]

    kt = _pack_segments(kall, sched)
    x = np.ascontiguousarray(np.asarray(x, dtype=np.float32))
    wei = np.ascontiguousarray(
        -np.abs(np.asarray(w_exc_inh, dtype=np.float32)).T)
    bng = np.asarray(bn_gamma, dtype=np.float32).reshape(NI, 1)
    bnb = np.asarray(bn_beta, dtype=np.float32).reshape(NI, 1)

    shared = {"kt": kt, "wei": wei, "bng": bng, "bnb": bnb}
    in_maps = []
    for c in range(N_CORES):
        m = dict(shared)
        m["xs"] = np.ascontiguousarray(x[c * BL:(c + 1) * BL])
        in_maps.append(m)

    _CACHE["in_maps"] = in_maps
    res = bass_utils.run_bass_kernel_spmd(nc, in_maps,
                                          core_ids=list(range(N_CORES)))
    out = np.concatenate([res.results[c]["out"] for c in range(N_CORES)],
                         axis=0)
    return out.astype(np.float32)


# revision 28
# speedup vs baseline: 1.1834x; 1.0180x over previous
# Trainium2 Bass kernel for nn_DCLS_semi_DANNLayer (DCLS gaussian convs + BN +
# LIF scan + inhibitory linear), data-parallel over batch on 8 NeuronCores.
#
# v3: host-built exact DCLS kernels; data-dependent tap skipping with an error
# budget; tail-chunk (60 ch) packs two taps per matmul via a shifted x copy;
# x is loaded CONTIGUOUSLY (5 channels per partition, kernel rows permuted on
# host to match) so DMA descriptors are 6KB instead of 1.2KB; kt arrives in
# 2-3 large per-sweep transfers; the inhibitory linear for the second exc
# slice accumulates (negated weights) directly into the conv PSUM so drains
# emit final outputs; PSUM drains run on the Scalar engine, BN + LIF scan on
# Vector, shadowed under the exc sweeps.
#
# Self-contained: hardcodes all shapes; takes FULL inputs, returns FULL output.
import numpy as np

import concourse.bacc as bacc
import concourse.bass as bass
import concourse.mybir as mybir
import concourse.tile as tile
from concourse import bass_utils


# ---- problem constants (hardcoded per spec) ----
N_CORES = 8
B, CI, T = 64, 700, 300
D = 25
TP = T - D + 1            # 276
NE, NI = 256, 128
BL = B // N_CORES         # 8 batches per core
N_LOC = BL * TP           # 2208, (b, t) layout
TAU = 2.0
A_DECAY = 1.0 - 1.0 / TAU  # 0.5
VTH = 1.0
BN_EPS = 1e-5
LIM = D // 2              # 12
TS = 256                  # per-batch columns in the paired matmul
TR = TP - TS              # 20 tail columns

N_CHUNK = 6               # ch0..ch4 (5-packed channels 0:640) + tail (640:700)
ROWS = [128, 128, 128, 128, 128, 120]

BUDGET_EXC = 0.04         # abs std of dropped-tap noise (output absmax ~100)
BUDGET_INH = 0.01

F32 = mybir.dt.float32
F32R = mybir.dt.float32r
ALU = mybir.AluOpType
ACTF = mybir.ActivationFunctionType

_CACHE: dict = {}


# ---------------------------------------------------------------- host side
def _build_dcls_host(W, P, SIG):
    """Exact DCLS 'gauss' kernel, matching the reference math. (O,I,1)->(O,I,D)"""
    j = np.arange(D, dtype=np.float32)
    Pc = np.clip(P[:, :, 0], -LIM, LIM).astype(np.float32) + np.float32(LIM)
    sig = np.abs(SIG[:, :, 0]).astype(np.float32) + np.float32(0.27)
    g = np.exp(np.float32(-0.5) * ((j[None, None, :] - Pc[..., None]) / sig[..., None]) ** 2)
    g = g / (g.sum(-1, keepdims=True) + np.float32(1e-7))
    return np.abs(W[:, :, 0]).astype(np.float32)[..., None] * g


def _tap_range(k, budget):
    """Minimal contiguous tap window [d0, d0+L) such that for every output
    channel the dropped-tap noise std (x ~ N(0,1)) is within budget."""
    var_od = (k.astype(np.float64) ** 2).sum(1)       # (O, D)
    total = var_od.sum(1)                             # (O,)
    for L in range(2, D + 1):
        for d0 in range(0, D - L + 1):
            dropped = total - var_od[:, d0:d0 + L].sum(1)
            if dropped.max() <= budget * budget:
                return d0, L
    return 0, D


def _sweep_width(L):
    return (5 * L + (L + 1) // 2) * 128


def _pack_segments(kall, sched):
    """kall: (384, 700, D) with exc rows 0:256, inh rows 256:384.
    Per-sweep contiguous layout [tail pairs | ch0 taps | .. | ch4 taps],
    chunk ch rows r hold channel 5r+ch (matching the contiguous x load)."""
    widths = [_sweep_width(L) for (_, _, L) in sched]
    kt = np.zeros((128, sum(widths)), dtype=np.float32)
    base = 0
    perm = 5 * np.arange(128)
    for s, (o0, d0, L) in enumerate(sched):
        taps = list(range(d0, d0 + L))
        npairs = (L + 1) // 2
        ev = taps[0::2]
        od = taps[1::2]
        buf = np.zeros((128, npairs, 128), dtype=np.float32)
        buf[0:60] = np.transpose(kall[o0:o0 + 128, 640:700, ev], (1, 2, 0))
        if od:
            buf[60:120, :len(od)] = np.transpose(
                kall[o0:o0 + 128, 640:700, od], (1, 2, 0))
        kt[:, base:base + npairs * 128] = buf.reshape(128, npairs * 128)
        for ch in range(5):
            blk = kall[o0:o0 + 128][:, perm + ch, :][:, :, d0:d0 + L]
            blk = np.transpose(blk, (1, 2, 0))        # (i, tap, o)
            off = base + (npairs + ch * L) * 128
            kt[:, off:off + L * 128] = blk.reshape(128, L * 128)
        base += widths[s]
    return kt


# ---------------------------------------------------------------- device side
def _build_nc(sched):
    nc = bacc.Bacc("TRN2", target_bir_lowering=False, debug=False,
                   num_devices=N_CORES)

    widths = [_sweep_width(L) for (_, _, L) in sched]
    bases = [sum(widths[:s]) for s in range(3)]
    ktw = max(widths)

    xs_d = nc.dram_tensor("xs", [BL, CI, T], F32R, kind="ExternalInput")
    kt_d = nc.dram_tensor("kt", [128, sum(widths)], F32R, kind="ExternalInput")
    wei_d = nc.dram_tensor("wei", [NI, NE], F32R, kind="ExternalInput")
    bng_d = nc.dram_tensor("bng", [NI, 1], F32, kind="ExternalInput")
    bnb_d = nc.dram_tensor("bnb", [NI, 1], F32, kind="ExternalInput")
    # o-major output layout: per-partition rows are BL*TP*4B contiguous, so
    # stores need only 128 large descriptors; host transposes back to (B,O,T)
    out_d = nc.dram_tensor("out", [NE, BL, TP], F32, kind="ExternalOutput")

    sw_taps = [list(range(d0, d0 + L)) for (_, d0, L) in sched]
    sw_npairs = [(L + 1) // 2 for (_, _, L) in sched]

    with tile.TileContext(nc) as tc:
        import contextlib

        with contextlib.ExitStack() as ctx:
            singles = ctx.enter_context(tc.tile_pool(name="singles", bufs=1))
            ktpool = ctx.enter_context(tc.tile_pool(name="ktpool", bufs=2))
            dpool = ctx.enter_context(
                tc.tile_pool(name="drampool", bufs=1, space="DRAM"))
            ppool = ctx.enter_context(
                tc.tile_pool(name="ppool", bufs=4, space="PSUM"))
            tpool = ctx.enter_context(
                tc.tile_pool(name="tpool", bufs=2, space="PSUM"))
            lpool = ctx.enter_context(
                tc.tile_pool(name="lpool", bufs=2, space="PSUM"))

            # ---- persistent SBUF tiles ----
            xtm = singles.tile([128, BL, 5, T], F32R)   # channels 0:640, 5/part
            xt5 = singles.tile([128, BL, T], F32R)      # channels 640:700 + shift
            inh = singles.tile([NI, N_LOC], F32)        # (b, t) layout
            inh3 = inh.rearrange("p (b t) -> p b t", t=TP)
            spk = singles.tile([NI, N_LOC], F32R)
            exc0 = singles.tile([128, BL, TP], F32)
            exc1 = singles.tile([128, BL, TP], F32)
            wei_neg = singles.tile([NI, NE], F32R)      # -|w_exc_inh|.T (host)
            bng = singles.tile([NI, 1], F32)
            bnb = singles.tile([NI, 1], F32)
            stats = singles.tile([NI, 4], F32)
            gst = singles.tile([NI, 2], F32)
            smalls = singles.tile([NI, 8], F32)
            w_st = singles.tile([NI, BL], F32)

            cc_in = dpool.tile([NI, 2], F32)
            cc_out = dpool.tile([NI, 2], F32, addr_space="Shared")

            kt_tiles = []

            def load_kt(s, splits):
                t_ = ktpool.tile([128, ktw], F32R, tag="kt", name=f"kt{s}")
                kt_tiles.append(t_)
                w = widths[s]
                cuts = [0] + splits + [w]
                for a, b_ in zip(cuts[:-1], cuts[1:]):
                    nc.sync.dma_start(out=t_[:, a:b_],
                                      in_=kt_d.ap()[:, bases[s] + a:bases[s] + b_])

            # ---- head DMAs (sync engine; order = priority) ----
            # sweep 0 starts with the tail chunk, which needs only the tail
            # kt columns (~0.5MB) and the small xt5 batches 0-3 (~0.3MB)
            L0 = len(sw_taps[0])
            P0 = sw_npairs[0]
            t0_ = ktpool.tile([128, ktw], F32R, tag="kt", name="kt0")
            kt_tiles.append(t0_)

            def kt_piece(t_, s, a, b_):
                nc.sync.dma_start(out=t_[:, a:b_],
                                  in_=kt_d.ap()[:, bases[s] + a:bases[s] + b_])

            def load_x(b_):
                nc.sync.dma_start(out=xtm[:, b_], in_=xs_d.ap()[b_, 0:640]
                                  .rearrange("(p c) t -> p c t", c=5))

            def load_x5(b_):
                # rows 60:120 (the one-tap-shifted copy) are filled by a fast
                # SBUF->SBUF DMA after the HBM loads land.
                nc.sync.dma_start(out=xt5[0:60, b_], in_=xs_d.ap()[b_, 640:700])

            kt_piece(t0_, 0, 0, P0 * 128)                       # s0 tail cols
            for b_ in range(4):
                load_x5(b_)
            nc.sync.dma_start(out=xt5[60:120, 0:4, 0:T - 1],
                              in_=xt5[0:60, 0:4, 1:T])
            kt_piece(t0_, 0, P0 * 128, (P0 + L0) * 128)         # s0 ch0
            for b_ in range(4):
                load_x(b_)
            for c_ in range(1, 5):                              # s0 ch1..ch4
                kt_piece(t0_, 0, (P0 + c_ * L0) * 128,
                         (P0 + (c_ + 1) * L0) * 128)
            for b_ in range(4, BL):
                load_x5(b_)
                load_x(b_)
            nc.sync.dma_start(out=xt5[60:120, 4:8, 0:T - 1],
                              in_=xt5[0:60, 4:8, 1:T])
            nc.sync.dma_start(out=wei_neg[:], in_=wei_d.ap())
            nc.sync.dma_start(out=bng[:], in_=bng_d.ap())
            nc.sync.dma_start(out=bnb[:], in_=bnb_d.ap())
            load_kt(1, [(sw_npairs[1] + len(sw_taps[1])) * 128])
            load_kt(2, [(sw_npairs[2] + len(sw_taps[2])) * 128])

            nc.vector.memset(w_st[:], 0.0)
            eps_c = smalls[:, 7:8]
            nc.vector.memset(eps_c, BN_EPS)

            def rhs(c, b0, nb, t0, t1):
                if c < 5:
                    return xtm[:, b0:b0 + nb, c, t0:t1]
                return xt5[:120, b0:b0 + nb, t0:t1]

            # ---- sweep emitter ----
            def emit_sweep(s, dst3, act_mid=None, dve_after_quad=None,
                           fused_lin=False, pe_tail=None, post_drain=None):
                taps = sw_taps[s]
                npairs = sw_npairs[s]
                L = len(taps)
                kt_t = kt_tiles[s]
                quads = []
                for q in range(2):
                    bA = 4 * q
                    pA = ppool.tile([128, 2, TS], F32, tag="pp", name=f"pA{s}{q}")
                    pB = ppool.tile([128, 2, TS], F32, tag="pp", name=f"pB{s}{q}")
                    tt = tpool.tile([128, 4, TR], F32, tag="tp", name=f"tt{s}{q}")
                    first = {id(pA): True, id(pB): True, id(tt): True}
                    for c in (5, 0, 1, 2, 3, 4):
                        r = ROWS[c]
                        n_units = L if c < 5 else npairs
                        off = (npairs + c * L) * 128 if c < 5 else 0
                        # per-tile grouping: pA's matmuls need only batches
                        # bA,bA+1 — they run while later batches still stream
                        for pt, b0, nb, lo, hi in (
                                (pA, bA, 2, 0, TS), (pB, bA + 2, 2, 0, TS),
                                (tt, bA, 4, TS, TP)):
                            for j in range(n_units):
                                lhsT = kt_t[:r,
                                            off + j * 128: off + (j + 1) * 128]
                                d = taps[j] if c < 5 else taps[2 * j]
                                last = (c == 4 and j == n_units - 1
                                        and not fused_lin)
                                nc.tensor.matmul(
                                    pt[:], lhsT, rhs(c, b0, nb, d + lo, d + hi),
                                    start=first[id(pt)], stop=last)
                                first[id(pt)] = False
                    quads.append((bA, pA, pB, tt))
                    if fused_lin:
                        continue
                    # drains on the Scalar engine
                    nc.scalar.copy(out=dst3[:, bA:bA + 2, 0:TS], in_=pA[:])
                    nc.scalar.copy(out=dst3[:, bA + 2:bA + 4, 0:TS], in_=pB[:])
                    nc.scalar.copy(out=dst3[:, bA:bA + 4, TS:TP], in_=tt[:])
                    if q == 0 and act_mid is not None:
                        act_mid()
                    if dve_after_quad is not None:
                        dve_after_quad(q)
                if not fused_lin:
                    return
                # Spike-dependent work only after ALL convs (PE is in-order:
                # a stall here cannot block any conv work).
                lw = wei_neg[:, 128:256]
                for bA, pA, pB, tt in quads:
                    # -|w|.T @ spikes accumulates into the conv PSUM so the
                    # drain emits final output values.
                    for i, pt in ((0, pA), (1, pB)):
                        for k in range(2):
                            b_ = bA + 2 * i + k
                            nc.tensor.matmul(
                                pt[:, k:k + 1, :], lw,
                                spk[:, b_ * TP:b_ * TP + TS],
                                start=False, stop=(k == 1),
                                skip_group_check=True)
                    for k in range(4):
                        b_ = bA + k
                        nc.tensor.matmul(
                            tt[:, k:k + 1, :], lw,
                            spk[:, b_ * TP + TS:(b_ + 1) * TP],
                            start=False, stop=(k == 3),
                            skip_group_check=True)
                for bA, pA, pB, tt in quads:
                    nc.scalar.copy(out=dst3[:, bA:bA + 2, 0:TS], in_=pA[:])
                    nc.scalar.copy(out=dst3[:, bA + 2:bA + 4, 0:TS], in_=pB[:])
                    nc.scalar.copy(out=dst3[:, bA:bA + 4, TS:TP], in_=tt[:])
                    if post_drain is not None:
                        post_drain(bA)
                if pe_tail is not None:
                    pe_tail()

            # ---------- sweep 0: inhibitory ----------
            def inh_stats(q):
                lo, hi = q * 4 * TP, (q + 1) * 4 * TP
                nc.vector.reduce_sum(stats[:, 2 * q:2 * q + 1], inh[:, lo:hi],
                                     axis=mybir.AxisListType.X)
                nc.vector.scalar_tensor_tensor(
                    spk[:, lo:hi], inh[:, lo:hi], 0.0, inh[:, lo:hi],
                    ALU.bypass, ALU.mult,
                    accum_out=stats[:, 2 * q + 1:2 * q + 2])

            emit_sweep(0, inh3, dve_after_quad=inh_stats)
            nc.vector.tensor_add(stats[:, 0:2], stats[:, 0:2], stats[:, 2:4])
            nc.scalar.dma_start(out=cc_in, in_=stats[:, 0:2])
            nc.gpsimd.collective_compute(
                "AllReduce", ALU.add,
                ins=[cc_in], outs=[cc_out],
                replica_groups=[list(range(N_CORES))],
            )

            # ---------- sweep 1: excitatory 0:128 ----------
            emit_sweep(1, exc0)

            # BN math AFTER all sweep-1 drains: a late collective must not
            # block the ACT drain stream (sweep-2 PSUM reuse depends on it).
            sg = smalls[:, 4:5]
            b2 = smalls[:, 6:7]
            nc.scalar.dma_start(out=gst[:], in_=cc_out)
            ninv = 1.0 / (N_LOC * N_CORES)
            nc.vector.tensor_scalar_mul(gst[:], gst[:], ninv)
            gmean = gst[:, 0:1]
            gex2 = gst[:, 1:2]
            msq = smalls[:, 0:1]
            nc.vector.tensor_mul(msq, gmean, gmean)
            var = smalls[:, 1:2]
            nc.vector.tensor_sub(var, gex2, msq)
            stdv = smalls[:, 2:3]
            nc.scalar.activation(stdv, var, ACTF.Sqrt, bias=eps_c)
            rstd = smalls[:, 3:4]
            nc.vector.reciprocal(rstd, stdv)
            nc.vector.tensor_mul(sg, rstd, bng[:])
            ms = smalls[:, 5:6]
            nc.vector.tensor_mul(ms, gmean, sg)
            nc.vector.tensor_sub(b2, bnb[:], ms)

            # ---------- BN apply + LIF scan (Vector, overlaps sweep 2) ----
            nc.vector.scalar_tensor_tensor(
                inh[:], inh[:], sg, b2.broadcast_to([NI, N_LOC]),
                ALU.mult, ALU.add)
            for t_i in range(TP):
                vsl = inh3[:, :, t_i]
                nc.vector.scalar_tensor_tensor(
                    vsl, w_st[:], A_DECAY, vsl, ALU.mult, ALU.add)
                nc.vector.scalar_tensor_tensor(
                    w_st[:], vsl, VTH, vsl, ALU.is_lt, ALU.mult)
            nc.vector.tensor_single_scalar(spk[:], inh[:], VTH, ALU.is_ge)

            # ---------- sweep 2: excitatory 128:256, lin fused ----------
            def lin_exc0():
                # exc0's linear via spare PSUM; add per batch, store per quad.
                # Output DMAs go on the ACT hwdge ring so they don't queue
                # behind or ahead of exc1's stores on the sync ring.
                lw = wei_neg[:, 0:128]
                for b_ in range(BL):
                    lp = lpool.tile([128, TP], F32, tag="lin", name=f"l0{b_}")
                    nc.tensor.matmul(lp[:], lw, spk[:, b_ * TP:(b_ + 1) * TP],
                                     start=True, stop=True)
                    nc.vector.tensor_add(
                        exc0[:, b_, :], exc0[:, b_, :], lp[:])
                    if b_ % 4 == 3:
                        nc.scalar.dma_start(
                            out=out_d.ap()[0:128, b_ - 3:b_ + 1, :],
                            in_=exc0[:, b_ - 3:b_ + 1, :])

            def store_exc1(bA):
                nc.sync.dma_start(out=out_d.ap()[128:256, bA:bA + 4, :],
                                  in_=exc1[:, bA:bA + 4, :])

            emit_sweep(2, exc1, fused_lin=True, pe_tail=lin_exc0,
                       post_drain=store_exc1)

    nc.compile()
    return nc


def kernel(x, W_inh, P_inh, SIG_inh, W_exc, P_exc, SIG_exc, w_exc_inh,
           bn_gamma, bn_beta):
    ke = _build_dcls_host(np.asarray(W_exc), np.asarray(P_exc),
                          np.asarray(SIG_exc))        # (256, 700, D)
    ki = _build_dcls_host(np.asarray(W_inh), np.asarray(P_inh),
                          np.asarray(SIG_inh))        # (128, 700, D)
    d0e, Le = _tap_range(ke, BUDGET_EXC)
    d0i, Li = _tap_range(ki, BUDGET_INH)
    kall = np.concatenate([ke, ki], axis=0)
    # sweeps: (o_offset into kall, d0, L) in order inh, exc0, exc1
    sched = ((256, d0i, Li), (0, d0e, Le), (128, d0e, Le))

    if _CACHE.get("key") != sched:
        _CACHE["nc"] = _build_nc(sched)
        _CACHE["key"] = sched
    nc = _CACHE["nc"]

    kt = _pack_segments(kall, sched)
    x = np.ascontiguousarray(np.asarray(x, dtype=np.float32))
    wei = np.ascontiguousarray(
        -np.abs(np.asarray(w_exc_inh, dtype=np.float32)).T)
    bng = np.asarray(bn_gamma, dtype=np.float32).reshape(NI, 1)
    bnb = np.asarray(bn_beta, dtype=np.float32).reshape(NI, 1)

    shared = {"kt": kt, "wei": wei, "bng": bng, "bnb": bnb}
    in_maps = []
    for c in range(N_CORES):
        m = dict(shared)
        m["xs"] = np.ascontiguousarray(x[c * BL:(c + 1) * BL])
        in_maps.append(m)

    _CACHE["in_maps"] = in_maps
    res = bass_utils.run_bass_kernel_spmd(nc, in_maps,
                                          core_ids=list(range(N_CORES)))
    # device emits (NE, BL, TP); transpose back to (BL, NE, TP) per core
    out = np.concatenate(
        [np.transpose(res.results[c]["out"], (1, 0, 2))
         for c in range(N_CORES)], axis=0)
    return np.ascontiguousarray(out, dtype=np.float32)


# revision 31
# speedup vs baseline: 1.1865x; 1.0026x over previous
# Trainium2 Bass kernel for nn_DCLS_semi_DANNLayer (DCLS gaussian convs + BN +
# LIF scan + inhibitory linear), data-parallel over batch on 8 NeuronCores.
#
# v3: host-built exact DCLS kernels; data-dependent tap skipping with an error
# budget; tail-chunk (60 ch) packs two taps per matmul via a shifted x copy;
# x is loaded CONTIGUOUSLY (5 channels per partition, kernel rows permuted on
# host to match) so DMA descriptors are 6KB instead of 1.2KB; kt arrives in
# 2-3 large per-sweep transfers; the inhibitory linear for the second exc
# slice accumulates (negated weights) directly into the conv PSUM so drains
# emit final outputs; PSUM drains run on the Scalar engine, BN + LIF scan on
# Vector, shadowed under the exc sweeps.
#
# Self-contained: hardcodes all shapes; takes FULL inputs, returns FULL output.
import numpy as np

import concourse.bacc as bacc
import concourse.bass as bass
import concourse.mybir as mybir
import concourse.tile as tile
from concourse import bass_utils


# ---- problem constants (hardcoded per spec) ----
N_CORES = 8
B, CI, T = 64, 700, 300
D = 25
TP = T - D + 1            # 276
NE, NI = 256, 128
BL = B // N_CORES         # 8 batches per core
N_LOC = BL * TP           # 2208, (b, t) layout
TAU = 2.0
A_DECAY = 1.0 - 1.0 / TAU  # 0.5
VTH = 1.0
BN_EPS = 1e-5
LIM = D // 2              # 12
TS = 256                  # per-batch columns in the paired matmul
TR = TP - TS              # 20 tail columns

N_CHUNK = 6               # ch0..ch4 (5-packed channels 0:640) + tail (640:700)
ROWS = [128, 128, 128, 128, 128, 120]

BUDGET_EXC = 0.04         # abs std of dropped-tap noise (output absmax ~100)
BUDGET_INH = 0.01

F32 = mybir.dt.float32
F32R = mybir.dt.float32r
ALU = mybir.AluOpType
ACTF = mybir.ActivationFunctionType

_CACHE: dict = {}


# ---------------------------------------------------------------- host side
def _build_dcls_host(W, P, SIG):
    """Exact DCLS 'gauss' kernel, matching the reference math. (O,I,1)->(O,I,D)"""
    j = np.arange(D, dtype=np.float32)
    Pc = np.clip(P[:, :, 0], -LIM, LIM).astype(np.float32) + np.float32(LIM)
    sig = np.abs(SIG[:, :, 0]).astype(np.float32) + np.float32(0.27)
    g = np.exp(np.float32(-0.5) * ((j[None, None, :] - Pc[..., None]) / sig[..., None]) ** 2)
    g = g / (g.sum(-1, keepdims=True) + np.float32(1e-7))
    return np.abs(W[:, :, 0]).astype(np.float32)[..., None] * g


def _tap_range(k, budget):
    """Minimal contiguous tap window [d0, d0+L) such that for every output
    channel the dropped-tap noise std (x ~ N(0,1)) is within budget."""
    var_od = (k.astype(np.float64) ** 2).sum(1)       # (O, D)
    total = var_od.sum(1)                             # (O,)
    for L in range(2, D + 1):
        for d0 in range(0, D - L + 1):
            dropped = total - var_od[:, d0:d0 + L].sum(1)
            if dropped.max() <= budget * budget:
                return d0, L
    return 0, D


def _sweep_width(L):
    return (5 * L + (L + 1) // 2) * 128


def _pack_segments(kall, sched):
    """kall: (384, 700, D) with exc rows 0:256, inh rows 256:384.
    Per-sweep contiguous layout [tail pairs | ch0 taps | .. | ch4 taps],
    chunk ch rows r hold channel 5r+ch (matching the contiguous x load)."""
    widths = [_sweep_width(L) for (_, _, L) in sched]
    kt = np.zeros((128, sum(widths)), dtype=np.float32)
    base = 0
    perm = 5 * np.arange(128)
    for s, (o0, d0, L) in enumerate(sched):
        taps = list(range(d0, d0 + L))
        npairs = (L + 1) // 2
        ev = taps[0::2]
        od = taps[1::2]
        buf = np.zeros((128, npairs, 128), dtype=np.float32)
        buf[0:60] = np.transpose(kall[o0:o0 + 128, 640:700, ev], (1, 2, 0))
        if od:
            buf[60:120, :len(od)] = np.transpose(
                kall[o0:o0 + 128, 640:700, od], (1, 2, 0))
        kt[:, base:base + npairs * 128] = buf.reshape(128, npairs * 128)
        for ch in range(5):
            blk = kall[o0:o0 + 128][:, perm + ch, :][:, :, d0:d0 + L]
            blk = np.transpose(blk, (1, 2, 0))        # (i, tap, o)
            off = base + (npairs + ch * L) * 128
            kt[:, off:off + L * 128] = blk.reshape(128, L * 128)
        base += widths[s]
    return kt


# ---------------------------------------------------------------- device side
def _build_nc(sched):
    nc = bacc.Bacc("TRN2", target_bir_lowering=False, debug=False,
                   num_devices=N_CORES)

    widths = [_sweep_width(L) for (_, _, L) in sched]
    bases = [sum(widths[:s]) for s in range(3)]
    ktw = max(widths)

    xs_d = nc.dram_tensor("xs", [BL, CI, T], F32R, kind="ExternalInput")
    kt_d = nc.dram_tensor("kt", [128, sum(widths)], F32R, kind="ExternalInput")
    wei_d = nc.dram_tensor("wei", [NI, NE], F32R, kind="ExternalInput")
    bng_d = nc.dram_tensor("bng", [NI, 1], F32, kind="ExternalInput")
    bnb_d = nc.dram_tensor("bnb", [NI, 1], F32, kind="ExternalInput")
    # o-major output layout: per-partition rows are BL*TP*4B contiguous, so
    # stores need only 128 large descriptors; host transposes back to (B,O,T)
    out_d = nc.dram_tensor("out", [NE, BL, TP], F32, kind="ExternalOutput")

    sw_taps = [list(range(d0, d0 + L)) for (_, d0, L) in sched]
    sw_npairs = [(L + 1) // 2 for (_, _, L) in sched]

    with tile.TileContext(nc) as tc:
        import contextlib

        with contextlib.ExitStack() as ctx:
            singles = ctx.enter_context(tc.tile_pool(name="singles", bufs=1))
            ktpool = ctx.enter_context(tc.tile_pool(name="ktpool", bufs=8))
            dpool = ctx.enter_context(
                tc.tile_pool(name="drampool", bufs=1, space="DRAM"))
            ppool = ctx.enter_context(
                tc.tile_pool(name="ppool", bufs=4, space="PSUM"))
            tpool = ctx.enter_context(
                tc.tile_pool(name="tpool", bufs=2, space="PSUM"))
            lpool = ctx.enter_context(
                tc.tile_pool(name="lpool", bufs=2, space="PSUM"))

            # ---- persistent SBUF tiles ----
            xtm = singles.tile([128, BL, 5, T], F32R)   # channels 0:640, 5/part
            xt5 = singles.tile([128, BL, T], F32R)      # channels 640:700 + shift
            inh = singles.tile([NI, N_LOC], F32)        # (b, t) layout
            inh3 = inh.rearrange("p (b t) -> p b t", t=TP)
            spk = singles.tile([NI, N_LOC], F32R)
            exc0 = singles.tile([128, BL, TP], F32)
            exc1 = singles.tile([128, BL, TP], F32)
            wei_neg = singles.tile([NI, NE], F32R)      # -|w_exc_inh|.T (host)
            bng = singles.tile([NI, 1], F32)
            bnb = singles.tile([NI, 1], F32)
            stats = singles.tile([NI, 4], F32)
            gst = singles.tile([NI, 2], F32)
            smalls = singles.tile([NI, 8], F32)
            w_st = singles.tile([NI, BL], F32)

            cc_in = dpool.tile([NI, 2], F32)
            cc_out = dpool.tile([NI, 2], F32, addr_space="Shared")

            # per-(sweep, chunk) kt tiles: tile-granular DMA dependencies so a
            # matmul only waits for its own chunk's columns
            kt_tiles = {}

            def load_seg(s, c):
                L = len(sw_taps[s])
                P = sw_npairs[s]
                ncols = (P if c == 5 else L) * 128
                a = 0 if c == 5 else (P + c * L) * 128
                t_ = ktpool.tile([128, 2048], F32R, tag="kt", name=f"kt{s}{c}")
                nc.sync.dma_start(
                    out=t_[:, :ncols],
                    in_=kt_d.ap()[:, bases[s] + a:bases[s] + a + ncols])
                kt_tiles[(s, c)] = t_

            # ---- head DMAs (sync engine; order = priority) ----
            # sweep 0 starts with the tail chunk, which needs only the tail
            # kt columns (~0.5MB) and the small xt5 batches 0-3 (~0.3MB)
            def load_x(b_):
                nc.sync.dma_start(out=xtm[:, b_], in_=xs_d.ap()[b_, 0:640]
                                  .rearrange("(p c) t -> p c t", c=5))

            def load_x5(b_):
                # rows 60:120 (the one-tap-shifted copy) are filled by a fast
                # SBUF->SBUF DMA after the HBM loads land.
                nc.sync.dma_start(out=xt5[0:60, b_], in_=xs_d.ap()[b_, 640:700])

            load_seg(0, 5)
            for b_ in range(4):
                load_x5(b_)
            nc.sync.dma_start(out=xt5[60:120, 0:4, 0:T - 1],
                              in_=xt5[0:60, 0:4, 1:T])
            load_x(0)
            load_x(1)
            load_seg(0, 0)
            load_x(2)
            load_x(3)
            load_seg(0, 1)
            for b_ in range(4, BL):
                load_x5(b_)
            nc.sync.dma_start(out=xt5[60:120, 4:8, 0:T - 1],
                              in_=xt5[0:60, 4:8, 1:T])
            for b_ in range(4, BL):
                load_x(b_)
            for c_ in (2, 3, 4):
                load_seg(0, c_)
            nc.sync.dma_start(out=wei_neg[:], in_=wei_d.ap())
            nc.sync.dma_start(out=bng[:], in_=bng_d.ap())
            nc.sync.dma_start(out=bnb[:], in_=bnb_d.ap())
            for s_ in (1, 2):
                for c_ in (5, 0, 1, 2, 3, 4):
                    load_seg(s_, c_)

            nc.vector.memset(w_st[:], 0.0)
            eps_c = smalls[:, 7:8]
            nc.vector.memset(eps_c, BN_EPS)

            def rhs(c, b0, nb, t0, t1):
                if c < 5:
                    return xtm[:, b0:b0 + nb, c, t0:t1]
                return xt5[:120, b0:b0 + nb, t0:t1]

            # ---- sweep emitter ----
            def emit_sweep(s, dst3, act_mid=None, dve_after_quad=None,
                           fused_lin=False, pe_tail=None, post_drain=None):
                taps = sw_taps[s]
                npairs = sw_npairs[s]
                L = len(taps)
                quads = []
                for q in range(2):
                    bA = 4 * q
                    pA = ppool.tile([128, 2, TS], F32, tag="pp", name=f"pA{s}{q}")
                    pB = ppool.tile([128, 2, TS], F32, tag="pp", name=f"pB{s}{q}")
                    tt = tpool.tile([128, 4, TR], F32, tag="tp", name=f"tt{s}{q}")
                    first = {id(pA): True, id(pB): True, id(tt): True}
                    for c in (5, 0, 1, 2, 3, 4):
                        r = ROWS[c]
                        kt_t = kt_tiles[(s, c)]
                        n_units = L if c < 5 else npairs
                        # per-tile grouping: pA's matmuls need only batches
                        # bA,bA+1 — they run while later batches still stream
                        for pt, b0, nb, lo, hi in (
                                (pA, bA, 2, 0, TS), (pB, bA + 2, 2, 0, TS),
                                (tt, bA, 4, TS, TP)):
                            for j in range(n_units):
                                lhsT = kt_t[:r, j * 128:(j + 1) * 128]
                                d = taps[j] if c < 5 else taps[2 * j]
                                last = (c == 4 and j == n_units - 1
                                        and not fused_lin)
                                nc.tensor.matmul(
                                    pt[:], lhsT, rhs(c, b0, nb, d + lo, d + hi),
                                    start=first[id(pt)], stop=last)
                                first[id(pt)] = False
                    quads.append((bA, pA, pB, tt))
                    if fused_lin:
                        continue
                    # drains on the Scalar engine
                    nc.scalar.copy(out=dst3[:, bA:bA + 2, 0:TS], in_=pA[:])
                    nc.scalar.copy(out=dst3[:, bA + 2:bA + 4, 0:TS], in_=pB[:])
                    nc.scalar.copy(out=dst3[:, bA:bA + 4, TS:TP], in_=tt[:])
                    if q == 0 and act_mid is not None:
                        act_mid()
                    if dve_after_quad is not None:
                        dve_after_quad(q)
                if not fused_lin:
                    return
                # Spike-dependent work only after ALL convs (PE is in-order:
                # a stall here cannot block any conv work).
                lw = wei_neg[:, 128:256]
                for bA, pA, pB, tt in quads:
                    # -|w|.T @ spikes accumulates into the conv PSUM so the
                    # drain emits final output values.
                    for i, pt in ((0, pA), (1, pB)):
                        for k in range(2):
                            b_ = bA + 2 * i + k
                            nc.tensor.matmul(
                                pt[:, k:k + 1, :], lw,
                                spk[:, b_ * TP:b_ * TP + TS],
                                start=False, stop=(k == 1),
                                skip_group_check=True)
                    for k in range(4):
                        b_ = bA + k
                        nc.tensor.matmul(
                            tt[:, k:k + 1, :], lw,
                            spk[:, b_ * TP + TS:(b_ + 1) * TP],
                            start=False, stop=(k == 3),
                            skip_group_check=True)
                for bA, pA, pB, tt in quads:
                    nc.scalar.copy(out=dst3[:, bA:bA + 2, 0:TS], in_=pA[:])
                    nc.scalar.copy(out=dst3[:, bA + 2:bA + 4, 0:TS], in_=pB[:])
                    nc.scalar.copy(out=dst3[:, bA:bA + 4, TS:TP], in_=tt[:])
                    if post_drain is not None:
                        post_drain(bA)
                if pe_tail is not None:
                    pe_tail()

            # ---------- sweep 0: inhibitory ----------
            def inh_stats(q):
                lo, hi = q * 4 * TP, (q + 1) * 4 * TP
                nc.vector.reduce_sum(stats[:, 2 * q:2 * q + 1], inh[:, lo:hi],
                                     axis=mybir.AxisListType.X)
                nc.vector.scalar_tensor_tensor(
                    spk[:, lo:hi], inh[:, lo:hi], 0.0, inh[:, lo:hi],
                    ALU.bypass, ALU.mult,
                    accum_out=stats[:, 2 * q + 1:2 * q + 2])

            emit_sweep(0, inh3, dve_after_quad=inh_stats)
            nc.vector.tensor_add(stats[:, 0:2], stats[:, 0:2], stats[:, 2:4])
            nc.scalar.dma_start(out=cc_in, in_=stats[:, 0:2])
            nc.gpsimd.collective_compute(
                "AllReduce", ALU.add,
                ins=[cc_in], outs=[cc_out],
                replica_groups=[list(range(N_CORES))],
            )

            # ---------- sweep 1: excitatory 0:128 ----------
            emit_sweep(1, exc0)

            # BN math AFTER all sweep-1 drains: a late collective must not
            # block the ACT drain stream (sweep-2 PSUM reuse depends on it).
            sg = smalls[:, 4:5]
            b2 = smalls[:, 6:7]
            nc.scalar.dma_start(out=gst[:], in_=cc_out)
            ninv = 1.0 / (N_LOC * N_CORES)
            nc.vector.tensor_scalar_mul(gst[:], gst[:], ninv)
            gmean = gst[:, 0:1]
            gex2 = gst[:, 1:2]
            msq = smalls[:, 0:1]
            nc.vector.tensor_mul(msq, gmean, gmean)
            var = smalls[:, 1:2]
            nc.vector.tensor_sub(var, gex2, msq)
            stdv = smalls[:, 2:3]
            nc.scalar.activation(stdv, var, ACTF.Sqrt, bias=eps_c)
            rstd = smalls[:, 3:4]
            nc.vector.reciprocal(rstd, stdv)
            nc.vector.tensor_mul(sg, rstd, bng[:])
            ms = smalls[:, 5:6]
            nc.vector.tensor_mul(ms, gmean, sg)
            nc.vector.tensor_sub(b2, bnb[:], ms)

            # ---------- BN apply + LIF scan (Vector, overlaps sweep 2) ----
            nc.vector.scalar_tensor_tensor(
                inh[:], inh[:], sg, b2.broadcast_to([NI, N_LOC]),
                ALU.mult, ALU.add)
            for t_i in range(TP):
                vsl = inh3[:, :, t_i]
                nc.vector.scalar_tensor_tensor(
                    vsl, w_st[:], A_DECAY, vsl, ALU.mult, ALU.add)
                nc.vector.scalar_tensor_tensor(
                    w_st[:], vsl, VTH, vsl, ALU.is_lt, ALU.mult)
            nc.vector.tensor_single_scalar(spk[:], inh[:], VTH, ALU.is_ge)

            # ---------- sweep 2: excitatory 128:256, lin fused ----------
            def lin_exc0():
                # exc0's linear via spare PSUM; add per batch, store per quad.
                # Output DMAs go on the ACT hwdge ring so they don't queue
                # behind or ahead of exc1's stores on the sync ring.
                lw = wei_neg[:, 0:128]
                for b_ in range(BL):
                    lp = lpool.tile([128, TP], F32, tag="lin", name=f"l0{b_}")
                    nc.tensor.matmul(lp[:], lw, spk[:, b_ * TP:(b_ + 1) * TP],
                                     start=True, stop=True)
                    nc.vector.tensor_add(
                        exc0[:, b_, :], exc0[:, b_, :], lp[:])
                    if b_ % 4 == 3:
                        nc.scalar.dma_start(
                            out=out_d.ap()[0:128, b_ - 3:b_ + 1, :],
                            in_=exc0[:, b_ - 3:b_ + 1, :])

            def store_exc1(bA):
                nc.sync.dma_start(out=out_d.ap()[128:256, bA:bA + 4, :],
                                  in_=exc1[:, bA:bA + 4, :])

            emit_sweep(2, exc1, fused_lin=True, pe_tail=lin_exc0,
                       post_drain=store_exc1)

    nc.compile()
    return nc


def kernel(x, W_inh, P_inh, SIG_inh, W_exc, P_exc, SIG_exc, w_exc_inh,
           bn_gamma, bn_beta):
    ke = _build_dcls_host(np.asarray(W_exc), np.asarray(P_exc),
                          np.asarray(SIG_exc))        # (256, 700, D)
    ki = _build_dcls_host(np.asarray(W_inh), np.asarray(P_inh),
                          np.asarray(SIG_inh))        # (128, 700, D)
    d0e, Le = _tap_range(ke, BUDGET_EXC)
    d0i, Li = _tap_range(ki, BUDGET_INH)
    kall = np.concatenate([ke, ki], axis=0)
    # sweeps: (o_offset into kall, d0, L) in order inh, exc0, exc1
    sched = ((256, d0i, Li), (0, d0e, Le), (128, d0e, Le))

    if _CACHE.get("key") != sched:
        _CACHE["nc"] = _build_nc(sched)
        _CACHE["key"] = sched
    nc = _CACHE["nc"]

    kt = _pack_segments(kall, sched)
    x = np.ascontiguousarray(np.asarray(x, dtype=np.float32))
    wei = np.ascontiguousarray(
        -np.abs(np.asarray(w_exc_inh, dtype=np.float32)).T)
    bng = np.asarray(bn_gamma, dtype=np.float32).reshape(NI, 1)
    bnb = np.asarray(bn_beta, dtype=np.float32).reshape(NI, 1)

    shared = {"kt": kt, "wei": wei, "bng": bng, "bnb": bnb}
    in_maps = []
    for c in range(N_CORES):
        m = dict(shared)
        m["xs"] = np.ascontiguousarray(x[c * BL:(c + 1) * BL])
        in_maps.append(m)

    _CACHE["in_maps"] = in_maps
    res = bass_utils.run_bass_kernel_spmd(nc, in_maps,
                                          core_ids=list(range(N_CORES)))
    # device emits (NE, BL, TP); transpose back to (BL, NE, TP) per core
    out = np.concatenate(
        [np.transpose(res.results[c]["out"], (1, 0, 2))
         for c in range(N_CORES)], axis=0)
    return np.ascontiguousarray(out, dtype=np.float32)


# revision 33
# speedup vs baseline: 1.4043x; 1.1836x over previous
# Trainium2 Bass kernel for nn_DCLS_semi_DANNLayer (DCLS gaussian convs + BN +
# LIF scan + inhibitory linear), data-parallel over batch on 8 NeuronCores.
#
# Design notes:
# - DCLS kernels are built exactly on the host and DMA'd in; taps whose
#   gaussian weight is negligible are skipped with a per-(branch, chunk)
#   error-budgeted window computed from the actual inputs at compile time.
# - x is loaded contiguously (5 channels per partition; kernel rows permuted
#   on host to match) so DMA descriptors are 6KB, not 1.2KB.
# - The leftover 60-channel chunk packs two taps per matmul via a one-tap-
#   shifted x copy in partitions 60:120 (filled by an SBUF->SBUF DMA).
# - Matmuls cover two batches x 256 t (512-col PSUM bank); the 20-col tails
#   run once per unit over all 8 batches. The inhibitory linear for the
#   second exc slice accumulates (negated weights) directly into conv PSUM.
# - PSUM drains run on the Scalar engine; BN + the 276-step LIF scan run on
#   Vector, shadowed under the exc sweeps; BN stats are all-reduced across
#   cores while the first exc sweep runs.
#
# Self-contained: hardcodes all shapes; takes FULL inputs, returns FULL output.
import numpy as np

import concourse.bacc as bacc
import concourse.bass as bass
import concourse.mybir as mybir
import concourse.tile as tile
from concourse import bass_utils


# ---- problem constants (hardcoded per spec) ----
N_CORES = 8
B, CI, T = 64, 700, 300
D = 25
TP = T - D + 1            # 276
NE, NI = 256, 128
BL = B // N_CORES         # 8 batches per core
N_LOC = BL * TP           # 2208, (b, t) layout
TAU = 2.0
A_DECAY = 1.0 - 1.0 / TAU  # 0.5
VTH = 1.0
BN_EPS = 1e-5
LIM = D // 2              # 12
TS = 256                  # per-batch columns in the paired matmul
TR = TP - TS              # 20 tail columns

N_CHUNK = 6               # ch0..ch4 (5-packed channels 0:640) + tail (640:700)
ROWS = [128, 128, 128, 128, 128, 120]
CH_ORDER = (5, 0, 1, 2, 3, 4)

BUDGET_EXC = 0.04         # abs std of dropped-tap noise (output absmax ~100)
BUDGET_INH = 0.01

F32 = mybir.dt.float32
F32R = mybir.dt.float32r
ALU = mybir.AluOpType
ACTF = mybir.ActivationFunctionType

_CACHE: dict = {}


# ---------------------------------------------------------------- host side
def _build_dcls_host(W, P, SIG):
    """Exact DCLS 'gauss' kernel, matching the reference math. (O,I,1)->(O,I,D)"""
    j = np.arange(D, dtype=np.float32)
    Pc = np.clip(P[:, :, 0], -LIM, LIM).astype(np.float32) + np.float32(LIM)
    sig = np.abs(SIG[:, :, 0]).astype(np.float32) + np.float32(0.27)
    g = np.exp(np.float32(-0.5) * ((j[None, None, :] - Pc[..., None]) / sig[..., None]) ** 2)
    g = g / (g.sum(-1, keepdims=True) + np.float32(1e-7))
    return np.abs(W[:, :, 0]).astype(np.float32)[..., None] * g


def _chunk_idx():
    perm = 5 * np.arange(128)
    return [perm + ch for ch in range(5)] + [np.arange(640, 700)]


def _chunk_windows(k, budget):
    """Per-chunk contiguous tap windows (d0, L) such that for every output
    channel the total dropped-tap noise std (x ~ N(0,1)) is within budget."""
    var = np.stack([(k[:, ix, :].astype(np.float64) ** 2).sum(1)
                    for ix in _chunk_idx()], 1)          # (O, 6, D)
    tot = var.sum(1)                                     # (O, D)
    total = tot.sum(1)                                   # (O,)
    B2 = budget * budget
    # minimal global window first
    g0, gL = 0, D
    for L in range(2, D + 1):
        done = False
        for d0 in range(0, D - L + 1):
            if (total - tot[:, d0:d0 + L].sum(1)).max() <= B2:
                g0, gL = d0, L
                done = True
                break
        if done:
            break
    win = [[g0, g0 + gL - 1] for _ in range(N_CHUNK)]
    V = total - tot[:, g0:g0 + gL].sum(1)
    # greedy per-chunk edge shrink
    while True:
        best = None
        for c in range(N_CHUNK):
            a, b_ = win[c]
            if b_ - a + 1 <= 1:
                continue
            for side, d in ((0, a), (1, b_)):
                vmax = (V + var[:, c, d]).max()
                if vmax <= B2 and (best is None or vmax < best[0]):
                    best = (vmax, c, side, d)
        if best is None:
            break
        _, c, side, d = best
        V = V + var[:, c, d]
        if side == 0:
            win[c][0] += 1
        else:
            win[c][1] -= 1
    return tuple((a, b_ - a + 1) for a, b_ in win)


def _sweep_layout(wins):
    """Column offsets per chunk within a sweep's kt block: [tail|ch0..ch4]."""
    np5 = (wins[5][1] + 1) // 2
    offs = [0] * N_CHUNK
    offs[5] = 0
    off = np5 * 128
    for c in range(5):
        offs[c] = off
        off += wins[c][1] * 128
    return offs, off


def _pack_segments(kall, sched):
    idxs = _chunk_idx()
    blocks = []
    for (o0, wins) in sched:
        offs, width = _sweep_layout(wins)
        blk = np.zeros((128, width), dtype=np.float32)
        d5, L5 = wins[5]
        taps5 = list(range(d5, d5 + L5))
        np5 = (L5 + 1) // 2
        buf = np.zeros((128, np5, 128), dtype=np.float32)
        buf[0:60] = np.transpose(kall[o0:o0 + 128][:, idxs[5], :]
                                 [:, :, taps5[0::2]], (1, 2, 0))
        if taps5[1::2]:
            buf[60:120, :len(taps5[1::2])] = np.transpose(
                kall[o0:o0 + 128][:, idxs[5], :][:, :, taps5[1::2]], (1, 2, 0))
        blk[:, 0:np5 * 128] = buf.reshape(128, np5 * 128)
        for c in range(5):
            d0, L = wins[c]
            sub = np.transpose(kall[o0:o0 + 128][:, idxs[c], d0:d0 + L],
                               (1, 2, 0))
            blk[:, offs[c]:offs[c] + L * 128] = sub.reshape(128, L * 128)
        blocks.append(blk)
    return np.concatenate(blocks, axis=1)


# ---------------------------------------------------------------- device side
def _build_nc(sched):
    nc = bacc.Bacc("TRN2", target_bir_lowering=False, debug=False,
                   num_devices=N_CORES)

    layouts = [_sweep_layout(wins) for (_, wins) in sched]
    widths = [w for (_, w) in layouts]
    bases = [sum(widths[:s]) for s in range(3)]

    xs_d = nc.dram_tensor("xs", [BL, CI, T], F32R, kind="ExternalInput")
    kt_d = nc.dram_tensor("kt", [128, sum(widths)], F32R, kind="ExternalInput")
    wei_d = nc.dram_tensor("wei", [NI, NE], F32R, kind="ExternalInput")
    bng_d = nc.dram_tensor("bng", [NI, 1], F32, kind="ExternalInput")
    bnb_d = nc.dram_tensor("bnb", [NI, 1], F32, kind="ExternalInput")
    # o-major output layout: per-partition rows are BL*TP*4B contiguous, so
    # stores need only 128 large descriptors; host transposes back to (B,O,T)
    out_d = nc.dram_tensor("out", [NE, BL, TP], F32, kind="ExternalOutput")

    def taps_of(s, c):
        d0, L = sched[s][1][c]
        return list(range(d0, d0 + L))

    with tile.TileContext(nc) as tc:
        import contextlib

        with contextlib.ExitStack() as ctx:
            singles = ctx.enter_context(tc.tile_pool(name="singles", bufs=1))
            ktpool = ctx.enter_context(tc.tile_pool(name="ktpool", bufs=10))
            dpool = ctx.enter_context(
                tc.tile_pool(name="drampool", bufs=1, space="DRAM"))
            ppool = ctx.enter_context(
                tc.tile_pool(name="ppool", bufs=4, space="PSUM"))
            tpool = ctx.enter_context(
                tc.tile_pool(name="tpool", bufs=2, space="PSUM"))
            lpool = ctx.enter_context(
                tc.tile_pool(name="lpool", bufs=2, space="PSUM"))

            # ---- persistent SBUF tiles ----
            xtm = singles.tile([128, BL, 5, T], F32R)   # channels 0:640, 5/part
            xt5 = singles.tile([128, BL, T], F32R)      # channels 640:700 + shift
            inh = singles.tile([NI, N_LOC], F32)        # (b, t) layout
            inh3 = inh.rearrange("p (b t) -> p b t", t=TP)
            spk = singles.tile([NI, N_LOC], F32R)
            exc0 = singles.tile([128, BL, TP], F32)
            exc1 = singles.tile([128, BL, TP], F32)
            wei_neg = singles.tile([NI, NE], F32R)      # -|w_exc_inh|.T (host)
            bng = singles.tile([NI, 1], F32)
            bnb = singles.tile([NI, 1], F32)
            stats = singles.tile([NI, 2], F32)
            gst = singles.tile([NI, 2], F32)
            smalls = singles.tile([NI, 8], F32)
            w_st = singles.tile([NI, BL], F32)

            cc_in = dpool.tile([NI, 2], F32)
            cc_out = dpool.tile([NI, 2], F32, addr_space="Shared")

            # per-(sweep, chunk) kt tiles: tile-granular DMA dependencies so a
            # matmul only waits for its own chunk's columns
            kt_tiles = {}

            ktw = max((((L_ + 1) // 2) if c_ == 5 else L_) * 128
                      for (_, ws) in sched for c_, (_, L_) in enumerate(ws))

            def load_seg(s, c):
                L = sched[s][1][c][1]
                ncols = (((L + 1) // 2) if c == 5 else L) * 128
                a = layouts[s][0][c]
                t_ = ktpool.tile([128, ktw], F32R, tag="kt", name=f"kt{s}{c}")
                nc.sync.dma_start(
                    out=t_[:, :ncols],
                    in_=kt_d.ap()[:, bases[s] + a:bases[s] + a + ncols])
                kt_tiles[(s, c)] = t_

            # ---- head DMAs (sync engine; order = priority) ----
            # sweep 0 starts with the tail chunk, which needs only the tail
            # kt columns (~0.5MB) and the small xt5 batches (~0.6MB)
            def load_x(b_):
                nc.sync.dma_start(out=xtm[:, b_], in_=xs_d.ap()[b_, 0:640]
                                  .rearrange("(p c) t -> p c t", c=5))

            def load_x5(b_):
                # rows 60:120 (the one-tap-shifted copy) are filled by a fast
                # SBUF->SBUF DMA after the HBM loads land.
                nc.sync.dma_start(out=xt5[0:60, b_], in_=xs_d.ap()[b_, 640:700])

            load_seg(0, 5)
            for b_ in range(4):
                load_x5(b_)
            nc.sync.dma_start(out=xt5[60:120, 0:4, 0:T - 1],
                              in_=xt5[0:60, 0:4, 1:T])
            load_x(0)
            load_x(1)
            load_seg(0, 0)
            load_x(2)
            load_x(3)
            load_seg(0, 1)
            for b_ in range(4, BL):
                load_x5(b_)
            nc.sync.dma_start(out=xt5[60:120, 4:8, 0:T - 1],
                              in_=xt5[0:60, 4:8, 1:T])
            for b_ in range(4, BL):
                load_x(b_)
            for c_ in (2, 3, 4):
                load_seg(0, c_)
            nc.sync.dma_start(out=wei_neg[:], in_=wei_d.ap())
            nc.sync.dma_start(out=bng[:], in_=bng_d.ap())
            nc.sync.dma_start(out=bnb[:], in_=bnb_d.ap())
            for s_ in (1, 2):
                for c_ in CH_ORDER:
                    load_seg(s_, c_)

            nc.vector.memset(w_st[:], 0.0)
            eps_c = smalls[:, 7:8]
            nc.vector.memset(eps_c, BN_EPS)

            def rhs(c, b0, nb, t0, t1):
                if c < 5:
                    return xtm[:, b0:b0 + nb, c, t0:t1]
                return xt5[:120, b0:b0 + nb, t0:t1]

            # ---- sweep emitter ----
            def emit_sweep(s, dst3, fused_lin=False, pe_tail=None,
                           post_drain=None):
                quads = []
                # batch-pair matmuls, quad-phased (pair drains free PSUM at
                # mid-sweep for the next sweep)
                for q in range(2):
                    bA = 4 * q
                    pA = ppool.tile([128, 2, TS], F32, tag="pp", name=f"pA{s}{q}")
                    pB = ppool.tile([128, 2, TS], F32, tag="pp", name=f"pB{s}{q}")
                    first = {id(pA): True, id(pB): True}
                    for c in CH_ORDER:
                        r = ROWS[c]
                        kt_t = kt_tiles[(s, c)]
                        taps = taps_of(s, c)
                        n_units = len(taps) if c < 5 else (len(taps) + 1) // 2
                        for pt, b0 in ((pA, bA), (pB, bA + 2)):
                            for j in range(n_units):
                                lhsT = kt_t[:r, j * 128:(j + 1) * 128]
                                d = taps[j] if c < 5 else taps[2 * j]
                                last = (c == 4 and j == n_units - 1
                                        and not fused_lin)
                                nc.tensor.matmul(
                                    pt[:], lhsT, rhs(c, b0, 2, d, d + TS),
                                    start=first[id(pt)], stop=last)
                                first[id(pt)] = False
                    quads.append((bA, pA, pB))
                    if fused_lin:
                        continue
                    nc.scalar.copy(out=dst3[:, bA:bA + 2, 0:TS], in_=pA[:])
                    nc.scalar.copy(out=dst3[:, bA + 2:bA + 4, 0:TS], in_=pB[:])
                # 20-col tails: one matmul per unit over all 8 batches
                tt = tpool.tile([128, BL, TR], F32, tag="tp", name=f"tt{s}")
                first_t = True
                for c in CH_ORDER:
                    r = ROWS[c]
                    kt_t = kt_tiles[(s, c)]
                    taps = taps_of(s, c)
                    n_units = len(taps) if c < 5 else (len(taps) + 1) // 2
                    for j in range(n_units):
                        lhsT = kt_t[:r, j * 128:(j + 1) * 128]
                        d = taps[j] if c < 5 else taps[2 * j]
                        last = (c == 4 and j == n_units - 1 and not fused_lin)
                        nc.tensor.matmul(
                            tt[:], lhsT, rhs(c, 0, BL, d + TS, d + TP),
                            start=first_t, stop=last)
                        first_t = False
                if not fused_lin:
                    nc.scalar.copy(out=dst3[:, :, TS:TP], in_=tt[:])
                    return
                # Spike-dependent work only after ALL convs (PE is in-order:
                # a stall here cannot block any conv work).
                lw = wei_neg[:, 128:256]
                for bA, pA, pB in quads:
                    # -|w|.T @ spikes accumulates into the conv PSUM so the
                    # drain emits final output values.
                    for i, pt in ((0, pA), (1, pB)):
                        for k in range(2):
                            b_ = bA + 2 * i + k
                            nc.tensor.matmul(
                                pt[:, k:k + 1, :], lw,
                                spk[:, b_ * TP:b_ * TP + TS],
                                start=False, stop=(k == 1),
                                skip_group_check=True)
                for b_ in range(BL):
                    nc.tensor.matmul(
                        tt[:, b_:b_ + 1, :], lw,
                        spk[:, b_ * TP + TS:(b_ + 1) * TP],
                        start=False, stop=(b_ == BL - 1),
                        skip_group_check=True)
                if pe_tail is not None:
                    pe_tail()
                for bA, pA, pB in quads:
                    nc.scalar.copy(out=dst3[:, bA:bA + 2, 0:TS], in_=pA[:])
                    nc.scalar.copy(out=dst3[:, bA + 2:bA + 4, 0:TS], in_=pB[:])
                nc.scalar.copy(out=dst3[:, :, TS:TP], in_=tt[:])
                if post_drain is not None:
                    post_drain()

            # ---------- sweep 0: inhibitory ----------
            emit_sweep(0, inh3)
            nc.vector.reduce_sum(stats[:, 0:1], inh[:],
                                 axis=mybir.AxisListType.X)
            nc.vector.scalar_tensor_tensor(
                spk[:], inh[:], 0.0, inh[:], ALU.bypass, ALU.mult,
                accum_out=stats[:, 1:2])
            nc.scalar.dma_start(out=cc_in, in_=stats[:])
            nc.gpsimd.collective_compute(
                "AllReduce", ALU.add,
                ins=[cc_in], outs=[cc_out],
                replica_groups=[list(range(N_CORES))],
            )

            # ---------- sweep 1: excitatory 0:128 ----------
            emit_sweep(1, exc0)

            # BN math AFTER all sweep-1 drains: a late collective must not
            # block the ACT drain stream (sweep-2 PSUM reuse depends on it).
            sg = smalls[:, 4:5]
            b2 = smalls[:, 6:7]
            nc.scalar.dma_start(out=gst[:], in_=cc_out)
            ninv = 1.0 / (N_LOC * N_CORES)
            nc.vector.tensor_scalar_mul(gst[:], gst[:], ninv)
            gmean = gst[:, 0:1]
            gex2 = gst[:, 1:2]
            msq = smalls[:, 0:1]
            nc.vector.tensor_mul(msq, gmean, gmean)
            var = smalls[:, 1:2]
            nc.vector.tensor_sub(var, gex2, msq)
            stdv = smalls[:, 2:3]
            nc.scalar.activation(stdv, var, ACTF.Sqrt, bias=eps_c)
            rstd = smalls[:, 3:4]
            nc.vector.reciprocal(rstd, stdv)
            nc.vector.tensor_mul(sg, rstd, bng[:])
            ms = smalls[:, 5:6]
            nc.vector.tensor_mul(ms, gmean, sg)
            nc.vector.tensor_sub(b2, bnb[:], ms)

            # ---------- BN apply + LIF scan (Vector, overlaps sweep 2) ----
            nc.vector.scalar_tensor_tensor(
                inh[:], inh[:], sg, b2.broadcast_to([NI, N_LOC]),
                ALU.mult, ALU.add)
            for t_i in range(TP):
                vsl = inh3[:, :, t_i]
                nc.vector.scalar_tensor_tensor(
                    vsl, w_st[:], A_DECAY, vsl, ALU.mult, ALU.add)
                nc.vector.scalar_tensor_tensor(
                    w_st[:], vsl, VTH, vsl, ALU.is_lt, ALU.mult)
            nc.vector.tensor_single_scalar(spk[:], inh[:], VTH, ALU.is_ge)

            # ---------- sweep 2: excitatory 128:256, lin fused ----------
            def lin_exc0():
                # exc0's linear via spare PSUM; add per batch, store per quad.
                # Output DMAs go on the ACT hwdge ring so they don't queue
                # behind or ahead of exc1's stores on the sync ring.
                lw = wei_neg[:, 0:128]
                for b_ in range(BL):
                    lp = lpool.tile([128, TP], F32, tag="lin", name=f"l0{b_}")
                    nc.tensor.matmul(lp[:], lw, spk[:, b_ * TP:(b_ + 1) * TP],
                                     start=True, stop=True)
                    nc.vector.tensor_add(
                        exc0[:, b_, :], exc0[:, b_, :], lp[:])
                    if b_ % 4 == 3:
                        nc.scalar.dma_start(
                            out=out_d.ap()[0:128, b_ - 3:b_ + 1, :],
                            in_=exc0[:, b_ - 3:b_ + 1, :])

            def store_exc1():
                nc.sync.dma_start(out=out_d.ap()[128:256, :, :],
                                  in_=exc1[:])

            emit_sweep(2, exc1, fused_lin=True, pe_tail=lin_exc0,
                       post_drain=store_exc1)

    nc.compile()
    return nc


def kernel(x, W_inh, P_inh, SIG_inh, W_exc, P_exc, SIG_exc, w_exc_inh,
           bn_gamma, bn_beta):
    ke = _build_dcls_host(np.asarray(W_exc), np.asarray(P_exc),
                          np.asarray(SIG_exc))        # (256, 700, D)
    ki = _build_dcls_host(np.asarray(W_inh), np.asarray(P_inh),
                          np.asarray(SIG_inh))        # (128, 700, D)
    wins_e = _chunk_windows(ke, BUDGET_EXC)
    wins_i = _chunk_windows(ki, BUDGET_INH)
    kall = np.concatenate([ke, ki], axis=0)
    # sweeps: (o_offset into kall, per-chunk windows) in order inh, exc0, exc1
    sched = ((256, wins_i), (0, wins_e), (128, wins_e))

    if _CACHE.get("key") != sched:
        _CACHE["nc"] = _build_nc(sched)
        _CACHE["key"] = sched
    nc = _CACHE["nc"]

    kt = _pack_segments(kall, sched)
    x = np.ascontiguousarray(np.asarray(x, dtype=np.float32))
    wei = np.ascontiguousarray(
        -np.abs(np.asarray(w_exc_inh, dtype=np.float32)).T)
    bng = np.asarray(bn_gamma, dtype=np.float32).reshape(NI, 1)
    bnb = np.asarray(bn_beta, dtype=np.float32).reshape(NI, 1)

    shared = {"kt": kt, "wei": wei, "bng": bng, "bnb": bnb}
    in_maps = []
    for c in range(N_CORES):
        m = dict(shared)
        m["xs"] = np.ascontiguousarray(x[c * BL:(c + 1) * BL])
        in_maps.append(m)

    _CACHE["in_maps"] = in_maps
    res = bass_utils.run_bass_kernel_spmd(nc, in_maps,
                                          core_ids=list(range(N_CORES)))
    # device emits (NE, BL, TP); transpose back to (BL, NE, TP) per core
    out = np.concatenate(
        [np.transpose(res.results[c]["out"], (1, 0, 2))
         for c in range(N_CORES)], axis=0)
    return np.ascontiguousarray(out, dtype=np.float32)
